# revision 35
# baseline (speedup 1.0000x reference)
"""Trainium2 Bass kernel for nn_Attention_89670327206161.

Dense transformer attention block, B=8 S=4096 D=1024 H=16 (dh=64), fp32.
The reference contracts attention scores over the *sequence* axis:
    scores_h = K_h^T Q_h / sqrt(dh)   -> (dh, dh) per head
    P_h      = softmax(scores_h, axis=-1)
    out_h    = V_h @ P_h              -> (S, dh)
    out      = concat_h(out_h) @ Wo^T

Because P_h is position-independent, the whole pipeline collapses
algebraically (exactly, no approximation):
    G        = x^T x                      (1024x1024 Gram, symmetric)
    scores_h = Wk_h G Wq_h^T              (== K_h^T Q_h)
    M        = Wv^T blockdiag(P_h) Wo^T   (1024x1024)
    out      = x @ M

This does ~10.9e9 MACs/core instead of ~17.8e9 for the direct
projection route (Q/K/V/O GEMMs): G (symmetric-half) + A = G Wq^T +
pair-packed Wk reduction + small M build + one output GEMM.

Sharding: pure data parallelism over batch -- one batch element per
NeuronCore, no collectives.

dtypes: score path (x_seq, G, Wq, Wk, A, softmax) is fp32/f32r --
logits reach |142| so they need ~1e-4 relative accuracy.  The output
path (P, Wv, Wo^T, DS, M, x^T) is bf16: its ~0.3% relative error is
40x under the 2e-2 gate and halves DMA+SBUF there.

Phases (per core):
  1. G = x^T x: stream 32 seq-chunks of 128 in 4 superchunks; PSUM
     accumulates upper-triangular row-block strips (pass B cols
     512:1024 for rc 0..7, pass A cols 0:512 for rc 0..3), DVE adds
     into SBUF G; 22 lower blocks mirrored via PE transpose.
  2. A = G @ Wq^T chunkwise (PSUM->SBUF), each chunk immediately
     reduced into persistent pair-packed score PSUM via Wk^T.
  3. Per-head softmax (max-subtracted exp, row-normalized) -> block-
     diagonal P pairs (bf16).
  4. DS = blockdiag(P)^T-applied Wv rows; M = DS^T-reduce with Wo^T,
     cast bf16.
  5. out = x @ M: stream x^T bf16 in 8 seq-blocks, 32 output tiles,
     DMA to HBM.
"""

import numpy as np

HEADS = 16
B, S, D = 8, 4096, 1024
P = 128                  # partitions
NKC = D // P             # 8 feature chunks of 128
NSC = S // P             # 32 seq chunks of 128
SUPER = 8                # seq chunks per superchunk
NSUP = NSC // SUPER      # 4
NPAIR = HEADS // 2       # 8 head pairs -> 128-wide blocks
N_CORES = 8

# G row-block strips: (rc, c0, c1).  Pass B covers cols 512:1024,
# pass A cols 0:512.  rc3/rc7 take full 512-wide strips (same PE cost
# as the 128-wide remnant at the <256 fp32r penalty) so their lower
# blocks come out directly and need no mirror.
G_PASS_B = [(0, 512, 1024), (1, 512, 1024), (2, 512, 1024), (3, 512, 1024),
            (4, 512, 1024), (5, 640, 1024), (6, 768, 1024), (7, 512, 1024)]
G_PASS_A = [(0, 0, 512), (1, 128, 512), (2, 256, 512), (3, 0, 512)]
# lower-triangle blocks (r, c) still needing a transpose-mirror.
# Blocks whose source strip comes from pass B (r >= 4, source strip c)
# go first in c-major order: strip c's super-3 evictions land in rc
# order, so the PE's transposes start with minimal waiting; the three
# pass-A-sourced mirrors trail.
G_MIRRORS = ([(r, c) for c in range(7) for r in range(max(4, c + 1), NKC)
              if not (r == 7 and 4 <= c < 7)]
             + [(1, 0), (2, 0), (2, 1)])

_PROGRAM = None


def _ts(i, n):
    return slice(i * n, (i + 1) * n)


def _build_program(repeat=1):
    # repeat>1 unrolls the whole computation R times in one program --
    # only used by measurement scripts to amplify device time above the
    # axon RPC dispatch noise.  kernel() always uses repeat=1.
    import concourse.bacc as bacc
    import concourse.mybir as mybir
    import concourse.tile as tile

    f32 = mybir.dt.float32
    f32r = mybir.dt.float32r
    bf16 = mybir.dt.bfloat16
    EXP = mybir.ActivationFunctionType.Exp
    X = mybir.AxisListType.X

    nc = bacc.Bacc(trn_type="TRN2", debug=False, num_devices=N_CORES)

    xs_d = nc.dram_tensor("xs", [S, D], f32r, kind="ExternalInput")
    xT_d = nc.dram_tensor("xTb", [D, S], bf16, kind="ExternalInput")
    wqT_d = nc.dram_tensor("wqT", [D, D], f32r, kind="ExternalInput")
    wkT_d = nc.dram_tensor("wkT", [D, D], f32r, kind="ExternalInput")
    wv_d = nc.dram_tensor("wv", [D, D], bf16, kind="ExternalInput")
    woT_d = nc.dram_tensor("woT", [D, D], bf16, kind="ExternalInput")
    eye_d = nc.dram_tensor("eye", [P, P], f32r, kind="ExternalInput")
    out_d = nc.dram_tensor("out", [S, D], f32, kind="ExternalOutput")

    xs_ap = xs_d.ap()                                        # (4096, 1024)
    xTr = xT_d.ap().rearrange("(c p) s -> p c s", p=P)       # (128, 8, 4096)
    wqTr = wqT_d.ap().rearrange("(c p) m -> p c m", p=P)
    wkTr = wkT_d.ap().rearrange("(c p) m -> p c m", p=P)
    wvr = wv_d.ap().rearrange("(c p) d -> p c d", p=P)
    woTr = woT_d.ap().rearrange("(c p) j -> p c j", p=P)

    with tile.TileContext(nc) as tc:
     for _rep in range(repeat):
      with tc.tile_pool(name="L0", bufs=1) as L0:
        zero_sb = L0.tile([P, 512], f32r, tag="zero")
        eye_sb = L0.tile([P, P], f32r, tag="eye")
        m_tiles = [L0.tile([P, D], bf16, tag=f"m{dj}", name=f"m{dj}")
                   for dj in range(NKC)]
        nc.vector.memset(zero_sb[:].bitcast(f32), 0.0)
        nc.sync.dma_start(eye_sb[:], eye_d.ap())

        with tc.tile_pool(name="Lg", bufs=1) as Lg:
          # one tile per row-block: dependency tracking is tile-granular,
          # so consumers of strip rc wait only on rc's own evictions
          g_tiles = [Lg.tile([P, D], f32r, tag=f"g{rc}", name=f"g{rc}")
                     for rc in range(NKC)]
          with tc.tile_pool(name="Lwq", bufs=1) as Lwq:
            wq_sb = Lwq.tile([P, NKC, D], f32r, tag="wq")

            # HAM warm-up: spin the PE on zero matmuls (gated only on
            # the memset) while the first x chunks are still in flight.
            with tc.tile_pool(name="scr_ps", bufs=1, space="PSUM") as scr:
                w_ps = scr.tile([P, 512], f32, tag="w")
                for _ in range(5):
                    nc.tensor.matmul(
                        w_ps[:], zero_sb[:, 0:P], zero_sb[:],
                        start=True, stop=False, skip_group_check=True,
                    )

            # ---- phase 1: G = x^T x (upper triangle) ----
            # DMA issue order matters: the x stream is the critical
            # path at startup, so three superchunks queue ahead of any
            # weight transfer; wq is issued after super 3's fetch.
            with (
                tc.tile_pool(name="Lxs", bufs=3 * SUPER) as Lxs,
                tc.tile_pool(name="g_ps", bufs=1, space="PSUM") as gps,
            ):
                def fetch_super(sp):
                    ts = []
                    for i in range(SUPER):
                        t = Lxs.tile([P, D], f32r, tag="xs")
                        nc.sync.dma_start(
                            t[:], xs_ap[_ts(sp * SUPER + i, P), :])
                        ts.append(t)
                    return ts

                # Bank (tag) plan: pass B tags = (rc+4)%8, pass A tags
                # = rc.  Within each pass, matmuls are emitted for the
                # longest-freed banks first and evictions run in the
                # order the *next* pass needs its banks back, so the PE
                # never waits more than ~0.3us on a DVE eviction.
                supers = [fetch_super(0), fetch_super(1), fetch_super(2)]
                for sp in range(NSUP):
                    xs_tiles = supers[sp]
                    if sp + 3 < NSUP:
                        supers.append(fetch_super(sp + 3))
                    if sp == 0:
                        # on the sync queue AFTER super 3's chunks: the
                        # SWDGE queue would fire it immediately and the
                        # (serialized) DMA engines would stall the
                        # critical x stream behind a 4MB transfer
                        nc.sync.dma_start(wq_sb[:], wqTr)
                    for strips, tag_of, emit_order, evict_order in (
                        (G_PASS_B, lambda rc: (rc + 4) % 8,
                         (0, 1, 2, 3, 4, 5, 6, 7), (4, 5, 6, 7, 0, 1, 2, 3)),
                        (G_PASS_A, lambda rc: rc,
                         (0, 1, 2, 3), (0, 1, 2, 3)),
                    ):
                        by_rc = {rc: (c0, c1) for rc, c0, c1 in strips}
                        ps = {rc: gps.tile([P, 512], f32, tag=f"g{tag_of(rc)}",
                                           name=f"gps{tag_of(rc)}")
                              for rc in emit_order}
                        for i, xt in enumerate(xs_tiles):
                            for rc in emit_order:
                                c0, c1 = by_rc[rc]
                                nc.tensor.matmul(
                                    ps[rc][:, 0:c1 - c0],
                                    xt[:, _ts(rc, P)], xt[:, c0:c1],
                                    start=(i == 0), stop=(i == SUPER - 1),
                                )
                        for rc in evict_order:
                            c0, c1 = by_rc[rc]
                            if sp == 0:
                                nc.vector.tensor_copy(
                                    g_tiles[rc][:, c0:c1], ps[rc][:, 0:c1 - c0])
                            else:
                                nc.vector.tensor_add(
                                    g_tiles[rc][:, c0:c1],
                                    g_tiles[rc][:, c0:c1],
                                    ps[rc][:, 0:c1 - c0])

            # mirror the remaining lower-triangle blocks
            with tc.tile_pool(name="mir_ps", bufs=4, space="PSUM") as mps:
                for r, c in G_MIRRORS:
                    mt = mps.tile([P, P], f32r, tag="mir")
                    nc.tensor.transpose(
                        mt[:], g_tiles[c][:, _ts(r, P)], eye_sb[:])
                    nc.vector.tensor_copy(g_tiles[r][:, _ts(c, P)], mt[:])

            # ---- phases 2-4 ----
            with tc.tile_pool(name="Lxt", bufs=3) as Lxt:
              with tc.tile_pool(name="L3", bufs=1) as L3:
                # allocation order fixes the SBUF address each tile
                # reuses from the released xs zone, and with it the
                # WAR release time of its DMA: wk (needed first, at the
                # score reduction) goes LAST so it sits over the xs
                # slots whose readers finish two passes early.
                wv_sb = L3.tile([P, NKC, D], bf16, tag="wv")
                wo_sb = L3.tile([P, NKC, D], bf16, tag="wo")
                # per-pair blockdiag P tiles so DS matmuls unblock as
                # soon as their own pair's softmax lands
                p_tiles = []
                for pr in range(NPAIR):
                    pt = L3.tile([P, P], bf16, tag=f"p{pr}", name=f"p{pr}")
                    nc.vector.memset(pt[:], 0.0)
                    p_tiles.append(pt)
                # softmax scratch (lives outside the scores-PSUM scope:
                # normalize/divide run after scores_ps is released).
                # One tile per (e, hf) group -- dependency tracking is
                # tile-granular, so shared tiles would serialize every
                # group's chain behind the last writer.
                p_tmp = L3.tile([P, 512], f32, tag="ptmp")
                nbm = [L3.tile([P, 4], f32, tag=f"nbm{g}", name=f"nbm{g}")
                       for g in range(4)]
                den = [L3.tile([P, 4], f32, tag=f"den{g}", name=f"den{g}")
                       for g in range(4)]
                rec = [L3.tile([P, 4], f32, tag=f"rec{g}", name=f"rec{g}")
                       for g in range(4)]
                ds_sb = L3.tile([P, NPAIR, D], bf16, tag="ds")
                wk_sb = L3.tile([P, NKC, D], f32r, tag="wk")
                nc.gpsimd.dma_start(wk_sb[:], wkTr)
                nc.gpsimd.dma_start(wv_sb[:], wvr)
                nc.gpsimd.dma_start(wo_sb[:], woTr)

                with tc.tile_pool(name="sc_ps", bufs=1, space="PSUM") as scps:
                    scores_ps = scps.tile([P, NPAIR * 256], f32, tag="sc")
                    for i in range(4):
                        nc.tensor.matmul(
                            scores_ps[:, _ts(i, 512)],
                            zero_sb[:, 0:P], zero_sb[:],
                            start=True, stop=False, skip_group_check=True,
                        )

                    # A = G @ Wq^T chunkwise; each chunk feeds the
                    # pair-packed score reduction.  Software-pipelined
                    # one chunk ahead so score matmuls never wait on a
                    # fresh eviction.
                    with (
                        tc.tile_pool(name="Lab", bufs=2) as Lab,
                        tc.tile_pool(name="a_ps", bufs=2, space="PSUM") as aps,
                    ):
                        def emit_a(dc):
                            a_ps = aps.tile([P, D], f32, tag="aps")
                            for jc in range(NKC):
                                for h in range(2):
                                    nc.tensor.matmul(
                                        a_ps[:, _ts(h, 512)],
                                        g_tiles[jc][:, _ts(dc, P)],
                                        wq_sb[:, jc, _ts(h, 512)],
                                        start=(jc == 0), stop=(jc == NKC - 1),
                                    )
                            # evict pre-scaled by 1/sqrt(dh): exact
                            # (power of 2), and it drops the bias-scale
                            # hop from the softmax dependency chain
                            a_sb = Lab.tile([P, D], f32r, tag="ab")
                            nc.scalar.mul(a_sb[:, 0:512], a_ps[:, 0:512], 0.125)
                            nc.vector.tensor_scalar_mul(
                                a_sb[:, 512:D], a_ps[:, 512:D], 0.125)
                            return a_sb

                        def emit_scores(dc, a_sb):
                            for pr in range(NPAIR):
                                nc.tensor.matmul(
                                    scores_ps[:, _ts(pr, 256)],
                                    wk_sb[:, dc, _ts(pr, P)],
                                    a_sb[:, _ts(pr // 2, 256)],
                                    start=False, stop=False,
                                    skip_group_check=True,
                                )

                        prev = (0, emit_a(0))
                        for dc in range(1, NKC):
                            a_sb = emit_a(dc)
                            emit_scores(*prev)
                            prev = (dc, a_sb)
                        emit_scores(*prev)

                    # ---- softmax + DS, interleaved by pair parity e.
                    # The 16 valid (64x64) diag blocks sit at cols
                    # 512k+384e+64hf (pr=2k+e).  One strided 3D-AP
                    # reduce per (e, hf) group yields all 4 per-BLOCK
                    # maxes at once (per-block max subtraction is
                    # mandatory: block maxes within a group differ by
                    # >91, past f32 exp underflow -- a shared group max
                    # NaNs the weak block's denominator).  Scores arrive
                    # pre-scaled by 0.125 so the max feeds exp's bias
                    # directly; exp's accum_out emits each block's
                    # denominator for free.  Parity 0's DS matmuls run
                    # on the PE while parity 1's chains are still on
                    # DVE/ACT; the d_ps pool sits on the A-pool PSUM
                    # zone (free at score end), not the scores zone the
                    # exps read.
                    sc3 = scores_ps.rearrange("p (k c) -> p k c", c=512)
                    with tc.tile_pool(name="ds_ps", bufs=2, space="PSUM") as dsps:
                        for e in range(2):
                            for hf in range(2):
                                g = 2 * e + hf
                                rows = slice(64 * hf, 64 * hf + 64)
                                off = 384 * e + 64 * hf
                                nc.vector.reduce_max(
                                    nbm[g][rows, 0:4], sc3[rows, :, off:off + 64],
                                    axis=X, negate=True)
                            for k in range(4):
                                pr = 2 * k + e
                                for hf in range(2):
                                    g = 2 * e + hf
                                    rows = slice(64 * hf, 64 * hf + 64)
                                    off = 512 * k + 384 * e + 64 * hf
                                    nc.scalar.activation(
                                        p_tmp[rows, _ts(pr, 64)],
                                        scores_ps[rows, off:off + 64], EXP,
                                        bias=nbm[g][rows, k:k + 1],
                                        accum_out=den[g][rows, k:k + 1])
                            for k in range(4):
                                pr = 2 * k + e
                                for hf in range(2):
                                    g = 2 * e + hf
                                    rows = slice(64 * hf, 64 * hf + 64)
                                    nc.vector.reciprocal(
                                        rec[g][rows, k:k + 1],
                                        den[g][rows, k:k + 1])
                                    src = p_tmp[rows, _ts(pr, 64)]
                                    # split normalize across ACT & DVE
                                    # (ACT Copy takes a per-partition
                                    # scale AP)
                                    if hf == 0:
                                        nc.scalar.activation(
                                            p_tiles[pr][rows, _ts(hf, 64)],
                                            src,
                                            mybir.ActivationFunctionType.Copy,
                                            scale=rec[g][rows, k:k + 1])
                                    else:
                                        nc.vector.tensor_scalar_mul(
                                            p_tiles[pr][rows, _ts(hf, 64)],
                                            src, rec[g][rows, k:k + 1])
                                d_ps = dsps.tile([P, D], f32, tag="dsps")
                                for h in range(2):
                                    nc.tensor.matmul(
                                        d_ps[:, _ts(h, 512)],
                                        p_tiles[pr][:],
                                        wv_sb[:, pr, _ts(h, 512)],
                                        start=True, stop=True,
                                    )
                                nc.scalar.copy(
                                    ds_sb[:, pr, 0:512], d_ps[:, 0:512])
                                nc.vector.tensor_copy(
                                    ds_sb[:, pr, 512:D], d_ps[:, 512:D])

                # ---- phase 4b: M = DS^T Wo^T (scores PSUM banks are
                # free now; m_ps takes them) ----
                with tc.tile_pool(name="m_ps", bufs=2, space="PSUM") as mmps:
                    # prefetch the x^T stream for phase 5 (WAR-gated on
                    # the released xs region, so these overlap phase 4)
                    xt_tiles = []
                    for b in range(NSC // 4):
                        t = Lxt.tile([P, NKC, 512], bf16, tag="xt")
                        nc.gpsimd.dma_start(t[:], xTr[:, :, _ts(b, 512)])
                        xt_tiles.append(t)

                    for dj in range(NKC):
                        m_ps = mmps.tile([P, D], f32, tag="mps")
                        for pr in range(NPAIR):
                            for h in range(2):
                                nc.tensor.matmul(
                                    m_ps[:, _ts(h, 512)],
                                    ds_sb[:, pr, _ts(dj, P)],
                                    wo_sb[:, pr, _ts(h, 512)],
                                    start=(pr == 0), stop=(pr == NPAIR - 1),
                                )
                        nc.scalar.copy(m_tiles[dj][:, 0:512], m_ps[:, 0:512])
                        nc.vector.tensor_copy(m_tiles[dj][:, 512:D], m_ps[:, 512:D])

              # ---- phase 5: out = x @ M ----
              with (
                  tc.tile_pool(name="Lob", bufs=2) as Lob,
                  tc.tile_pool(name="o_ps", bufs=3, space="PSUM") as ops,
              ):
                  for st in range(NSC):
                      xt = xt_tiles[st // 4]
                      so = st % 4
                      o_ps = ops.tile([P, D], f32, tag="ops")
                      for dc in range(NKC):
                          for h in range(2):
                              nc.tensor.matmul(
                                  o_ps[:, _ts(h, 512)],
                                  xt[:, dc, _ts(so, P)],
                                  m_tiles[dc][:, _ts(h, 512)],
                                  start=(dc == 0), stop=(dc == NKC - 1),
                              )
                      o_sb = Lob.tile([P, D], f32, tag="ob")
                      if st < NSC - 1:
                          nc.scalar.copy(o_sb[:, 0:512], o_ps[:, 0:512])
                          nc.vector.tensor_copy(o_sb[:, 512:D], o_ps[:, 512:D])
                          nc.sync.dma_start(out_d.ap()[_ts(st, P), :], o_sb[:])
                      else:
                          # split the last tile so the end-of-program
                          # drain only waits on a 256-col evict + DMA
                          for q in range(4):
                              cs = _ts(q, 256)
                              if q % 2 == 0:
                                  nc.scalar.copy(o_sb[:, cs], o_ps[:, cs])
                              else:
                                  nc.vector.tensor_copy(o_sb[:, cs], o_ps[:, cs])
                              nc.sync.dma_start(
                                  out_d.ap()[_ts(st, P), 256 * q:256 * q + 256],
                                  o_sb[:, cs])

    nc.compile()
    return nc


def _get_program():
    global _PROGRAM
    if _PROGRAM is None:
        _PROGRAM = _build_program()
    return _PROGRAM


def _prep_in_maps(x, Wq, Wk, Wv, Wo):
    import ml_dtypes

    bf = ml_dtypes.bfloat16
    x_np = np.asarray(x, np.float32)
    wqT = np.ascontiguousarray(np.asarray(Wq, np.float32).T)
    wkT = np.ascontiguousarray(np.asarray(Wk, np.float32).T)
    wv = np.ascontiguousarray(np.asarray(Wv, np.float32)).astype(bf)
    woT = np.ascontiguousarray(np.asarray(Wo, np.float32).T).astype(bf)
    eye = np.eye(P, dtype=np.float32)
    in_maps = []
    for b in range(N_CORES):
        xs = np.ascontiguousarray(x_np[b])
        xTb = np.ascontiguousarray(x_np[b].T).astype(bf)
        in_maps.append({"xs": xs, "xTb": xTb, "wqT": wqT, "wkT": wkT,
                        "wv": wv, "woT": woT, "eye": eye})
    return in_maps


def kernel(x, Wq, Wk, Wv, Wo):
    from concourse import bass_utils

    nc = _get_program()
    in_maps = _prep_in_maps(x, Wq, Wk, Wv, Wo)
    res = bass_utils.run_bass_kernel_spmd(nc, in_maps, core_ids=list(range(N_CORES)))
    return np.stack([res.results[b]["out"] for b in range(N_CORES)], axis=0)


# revision 52
# speedup vs baseline: 3.6108x; 3.6108x over previous
"""Trainium2 Bass kernel for nn_Attention_89670327206161.

Dense transformer attention block, B=8 S=4096 D=1024 H=16 (dh=64), fp32.
The reference contracts attention scores over the *sequence* axis:
    scores_h = K_h^T Q_h / sqrt(dh)   -> (dh, dh) per head
    P_h      = softmax(scores_h, axis=-1)
    out_h    = V_h @ P_h              -> (S, dh)
    out      = concat_h(out_h) @ Wo^T

Because P_h is position-independent, the whole pipeline collapses
algebraically (exactly, no approximation):
    G        = x^T x                      (1024x1024 Gram, symmetric)
    scores_h = Wk_h G Wq_h^T              (== K_h^T Q_h)
    M        = Wv^T blockdiag(P_h) Wo^T   (1024x1024)
    out      = x @ M

This does ~10.9e9 MACs/core instead of ~17.8e9 for the direct
projection route (Q/K/V/O GEMMs): G (symmetric-half) + A = G Wq^T +
pair-packed Wk reduction + small M build + one output GEMM.

Sharding: pure data parallelism over batch -- one batch element per
NeuronCore, no collectives.

dtypes: score path (x_seq, G, Wq, Wk, A, softmax) is fp32/f32r --
logits reach |142| so they need ~1e-4 relative accuracy.  The output
path (P, Wv, Wo^T, DS, M, x^T) is bf16: its ~0.3% relative error is
40x under the 2e-2 gate and halves DMA+SBUF there.

Phases (per core):
  1. G = x^T x: stream 32 seq-chunks of 128 in 4 superchunks; PSUM
     accumulates upper-triangular row-block strips (pass B cols
     512:1024 for rc 0..7, pass A cols 0:512 for rc 0..3), DVE adds
     into SBUF G; 22 lower blocks mirrored via PE transpose.
  2. A = G @ Wq^T chunkwise (PSUM->SBUF), each chunk immediately
     reduced into persistent pair-packed score PSUM via Wk^T.
  3. Per-head softmax (max-subtracted exp, row-normalized) -> block-
     diagonal P pairs (bf16).
  4. DS = blockdiag(P)^T-applied Wv rows; M = DS^T-reduce with Wo^T,
     cast bf16.
  5. out = x @ M: stream x^T bf16 in 8 seq-blocks, 32 output tiles,
     DMA to HBM.
"""

import numpy as np

HEADS = 16
B, S, D = 8, 4096, 1024
P = 128                  # partitions
NKC = D // P             # 8 feature chunks of 128
NSC = S // P             # 32 seq chunks of 128
SUPER = 8                # seq chunks per superchunk
NSUP = NSC // SUPER      # 4
NPAIR = HEADS // 2       # 8 head pairs -> 128-wide blocks
N_CORES = 8

# G row-block strips: (rc, c0, c1).  Pass B covers cols 512:1024,
# pass A cols 0:512.  rc3/rc7 take full 512-wide strips (same PE cost
# as the 128-wide remnant at the <256 fp32r penalty) so their lower
# blocks come out directly and need no mirror.
G_PASS_B = [(0, 512, 1024), (1, 512, 1024), (2, 512, 1024), (3, 512, 1024),
            (4, 512, 1024), (5, 640, 1024), (6, 768, 1024), (7, 512, 1024)]
G_PASS_A = [(0, 0, 512), (1, 128, 512), (2, 256, 512), (3, 0, 512)]
# lower-triangle blocks (r, c) still needing a transpose-mirror.
# Blocks whose source strip comes from pass B (r >= 4, source strip c)
# go first in c-major order: strip c's super-3 evictions land in rc
# order, so the PE's transposes start with minimal waiting; the three
# pass-A-sourced mirrors trail.
G_MIRRORS = ([(r, c) for c in range(7) for r in range(max(4, c + 1), NKC)
              if not (r == 7 and 4 <= c < 7)]
             + [(1, 0), (2, 0), (2, 1)])

_PROGRAM = None


def _ts(i, n):
    return slice(i * n, (i + 1) * n)


def _build_program(repeat=1):
    # repeat>1 unrolls the whole computation R times in one program --
    # only used by measurement scripts to amplify device time above the
    # axon RPC dispatch noise.  kernel() always uses repeat=1.
    import concourse.bacc as bacc
    import concourse.mybir as mybir
    import concourse.tile as tile

    f32 = mybir.dt.float32
    f32r = mybir.dt.float32r
    bf16 = mybir.dt.bfloat16
    EXP = mybir.ActivationFunctionType.Exp
    X = mybir.AxisListType.X

    nc = bacc.Bacc(trn_type="TRN2", debug=False, num_devices=N_CORES)

    xs_d = nc.dram_tensor("xs", [S, D], f32r, kind="ExternalInput")
    xT_d = nc.dram_tensor("xTb", [D, S], bf16, kind="ExternalInput")
    wqT_d = nc.dram_tensor("wqT", [D, D], f32r, kind="ExternalInput")
    wkT_d = nc.dram_tensor("wkT", [D, D], f32r, kind="ExternalInput")
    wv_d = nc.dram_tensor("wv", [D, D], bf16, kind="ExternalInput")
    woT_d = nc.dram_tensor("woT", [D, D], bf16, kind="ExternalInput")
    eye_d = nc.dram_tensor("eye", [P, P], f32r, kind="ExternalInput")
    out_d = nc.dram_tensor("out", [S, D], f32, kind="ExternalOutput")

    xs_ap = xs_d.ap()                                        # (4096, 1024)
    xTr = xT_d.ap().rearrange("(c p) s -> p c s", p=P)       # (128, 8, 4096)
    wqTr = wqT_d.ap().rearrange("(c p) m -> p c m", p=P)
    wkTr = wkT_d.ap().rearrange("(c p) m -> p c m", p=P)
    wvr = wv_d.ap().rearrange("(c p) d -> p c d", p=P)
    woTr = woT_d.ap().rearrange("(c p) j -> p c j", p=P)

    with tile.TileContext(nc) as tc:
     for _rep in range(repeat):
      with tc.tile_pool(name="L0", bufs=1) as L0:
        zero_sb = L0.tile([P, 512], f32r, tag="zero")
        eye_sb = L0.tile([P, P], f32r, tag="eye")
        m_tiles = [L0.tile([P, D], bf16, tag=f"m{dj}", name=f"m{dj}")
                   for dj in range(NKC)]
        nc.vector.memset(zero_sb[:].bitcast(f32), 0.0)
        # prewarm the ACT function table during the initial DMA wait --
        # the first activation otherwise pays a ~1.3us LoadActFuncSet
        # right where the mirror copies start
        actw = L0.tile([P, 1], f32, tag="actw")
        nc.scalar.activation(actw[:], zero_sb[:, 0:1], EXP)

        with tc.tile_pool(name="Lg", bufs=1) as Lg:
          # one tile per (row-block, column-half): dependency tracking
          # is tile-granular, so pass-B consumers (all 19 lower-mirror
          # sources) never wait on the final pass-A evictions
          g_lo = [Lg.tile([P, 512], f32r, tag=f"glo{rc}", name=f"glo{rc}")
                  for rc in range(NKC)]
          g_hi = [Lg.tile([P, 512], f32r, tag=f"ghi{rc}", name=f"ghi{rc}")
                  for rc in range(NKC)]

          def g_block(jc, dc):
              # (128 x 128) AP of G[jc*128:(jc+1)*128, dc*128:(dc+1)*128]
              if dc < 4:
                  return g_lo[jc][:, _ts(dc, P)]
              return g_hi[jc][:, _ts(dc - 4, P)]
          with tc.tile_pool(name="Lwq", bufs=1) as Lwq:
            wq_sb = Lwq.tile([P, NKC, D], f32r, tag="wq")

            # HAM warm-up: spin the PE on zero matmuls (gated only on
            # the memset) while the first x chunks are still in flight.
            with tc.tile_pool(name="scr_ps", bufs=1, space="PSUM") as scr:
                w_ps = scr.tile([P, 512], f32, tag="w")
                for _ in range(8):
                    nc.tensor.matmul(
                        w_ps[:], zero_sb[:, 0:P], zero_sb[:],
                        start=True, stop=False, skip_group_check=True,
                    )

            # ---- phase 1: G = x^T x (upper triangle) ----
            # DMA issue order matters: the x stream is the critical
            # path at startup, so three superchunks queue ahead of any
            # weight transfer; wq is issued after super 3's fetch.
            with (
                tc.tile_pool(name="Lxs", bufs=3 * SUPER) as Lxs,
                tc.tile_pool(name="g_ps", bufs=1, space="PSUM") as gps,
            ):
                def fetch_super(sp):
                    ts = []
                    for i in range(SUPER):
                        t = Lxs.tile([P, D], f32r, tag="xs")
                        nc.sync.dma_start(
                            t[:], xs_ap[_ts(sp * SUPER + i, P), :])
                        ts.append(t)
                    return ts

                # Bank (tag) plan: pass B tags = (rc+4)%8, pass A tags
                # = rc.  Within each pass, matmuls are emitted for the
                # longest-freed banks first and evictions run in the
                # order the *next* pass needs its banks back, so the PE
                # never waits more than ~0.3us on a DVE eviction.
                supers = [fetch_super(0)]
                # eye is not needed until the mirrors (~78us): queue it
                # behind the first superchunk so chunk 0 lands sooner
                nc.sync.dma_start(eye_sb[:], eye_d.ap())
                supers += [fetch_super(1), fetch_super(2)]
                for sp in range(NSUP):
                    xs_tiles = supers[sp]
                    if sp + 3 < NSUP:
                        supers.append(fetch_super(sp + 3))
                    if sp == 0:
                        # on the sync queue AFTER super 3's chunks: the
                        # SWDGE queue would fire it immediately and the
                        # (serialized) DMA engines would stall the
                        # critical x stream behind a 4MB transfer
                        nc.sync.dma_start(wq_sb[:], wqTr)
                    for strips, tag_of, emit_order, evict_order in (
                        (G_PASS_B, lambda rc: (rc + 4) % 8,
                         (0, 1, 2, 3, 4, 5, 6, 7), (4, 5, 6, 7, 0, 1, 2, 3)),
                        (G_PASS_A, lambda rc: rc,
                         (0, 1, 2, 3), (0, 1, 2, 3)),
                    ):
                        by_rc = {rc: (c0, c1) for rc, c0, c1 in strips}
                        ps = {rc: gps.tile([P, 512], f32, tag=f"g{tag_of(rc)}",
                                           name=f"gps{tag_of(rc)}")
                              for rc in emit_order}
                        for i, xt in enumerate(xs_tiles):
                            for rc in emit_order:
                                c0, c1 = by_rc[rc]
                                nc.tensor.matmul(
                                    ps[rc][:, 0:c1 - c0],
                                    xt[:, _ts(rc, P)], xt[:, c0:c1],
                                    start=(i == 0), stop=(i == SUPER - 1),
                                )
                        for rc in evict_order:
                            c0, c1 = by_rc[rc]
                            if c0 >= 512:
                                dst = g_hi[rc][:, c0 - 512:c1 - 512]
                            else:
                                dst = g_lo[rc][:, c0:c1]
                            if sp == 0:
                                nc.vector.tensor_copy(dst, ps[rc][:, 0:c1 - c0])
                            else:
                                nc.vector.tensor_add(
                                    dst, dst, ps[rc][:, 0:c1 - c0])

            # mirror the remaining lower-triangle blocks: all of a
            # target (row, half)'s transposes land side-by-side in one
            # wide PSUM tile, evicted by a single wide copy (split
            # across ACT and DVE).  22 narrow copies would make the DVE
            # the bottleneck of this whole phase (~5.7us).  19 of the
            # 22 sources live in g_hi (pass B), whose super-3 evictions
            # complete during the final pass-A matmuls.
            with tc.tile_pool(name="mir_ps", bufs=1, space="PSUM") as mps:
                groups = {}
                for r, c in G_MIRRORS:
                    groups.setdefault((r, c // 4), []).append(c)
                # allocation order: late-needed tiles go first, so the
                # early transposes land on the PSUM banks the G pool's
                # pass-B tiles vacate first
                alloc = [(5, 1), (6, 1), (1, 0), (2, 0),
                         (4, 0), (5, 0), (6, 0), (7, 0)]
                mwide = {key: mps.tile([P, len(groups[key]) * P], f32r,
                                       tag=f"mir{key[0]}{key[1]}",
                                       name=f"mir{key[0]}{key[1]}")
                         for key in alloc}
                for r, c in G_MIRRORS:
                    key = (r, c // 4)
                    i = groups[key].index(c)
                    nc.tensor.matmul(
                        mwide[key][:, _ts(i, P)], g_block(c, r), eye_sb[:],
                        is_transpose=True,
                        start=True, stop=True, skip_group_check=True)
                for j, key in enumerate([(4, 0), (5, 0), (6, 0), (7, 0),
                                         (5, 1), (6, 1), (1, 0), (2, 0)]):
                    r, half = key
                    cs = groups[key]
                    c0 = min(cs) - 4 * half
                    gt = g_lo[r] if half == 0 else g_hi[r]
                    dst = gt[:, c0 * P:(c0 + len(cs)) * P]
                    if j % 2 == 0:
                        nc.scalar.copy(dst, mwide[key][:])
                    else:
                        nc.vector.tensor_copy(dst, mwide[key][:])

            # ---- phases 2-4 ----
            with tc.tile_pool(name="Lxt", bufs=3) as Lxt:
              with tc.tile_pool(name="L3", bufs=1) as L3:
                # allocation order fixes the SBUF address each tile
                # reuses from the released xs zone, and with it the
                # WAR release time of its DMA: wk (needed first, at the
                # score reduction) goes LAST so it sits over the xs
                # slots whose readers finish two passes early.
                wv_sb = L3.tile([P, NKC, D], bf16, tag="wv")
                wo_sb = L3.tile([P, NKC, D], bf16, tag="wo")
                # per-pair blockdiag P tiles so DS matmuls unblock as
                # soon as their own pair's softmax lands
                p_tiles = []
                for pr in range(NPAIR):
                    pt = L3.tile([P, P], bf16, tag=f"p{pr}", name=f"p{pr}")
                    nc.vector.memset(pt[:], 0.0)
                    p_tiles.append(pt)
                # softmax scratch (lives outside the scores-PSUM scope:
                # normalize/divide run after scores_ps is released).
                # One tile per (e, hf) group -- dependency tracking is
                # tile-granular, so shared tiles would serialize every
                # group's chain behind the last writer.
                p_tmp = L3.tile([P, 512], f32, tag="ptmp")
                nbm = [L3.tile([P, 4], f32, tag=f"nbm{g}", name=f"nbm{g}")
                       for g in range(4)]
                den = [L3.tile([P, 4], f32, tag=f"den{g}", name=f"den{g}")
                       for g in range(4)]
                rec = [L3.tile([P, 4], f32, tag=f"rec{g}", name=f"rec{g}")
                       for g in range(4)]
                ds_sb = L3.tile([P, NPAIR, D], bf16, tag="ds")
                wk_sb = L3.tile([P, NKC, D], f32r, tag="wk")
                nc.gpsimd.dma_start(wk_sb[:], wkTr)
                nc.gpsimd.dma_start(wv_sb[:], wvr)
                nc.gpsimd.dma_start(wo_sb[:], woTr)

                with tc.tile_pool(name="sc_ps", bufs=1, space="PSUM") as scps:
                    scores_ps = scps.tile([P, NPAIR * 256], f32, tag="sc")
                    for i in range(4):
                        nc.tensor.matmul(
                            scores_ps[:, _ts(i, 512)],
                            zero_sb[:, 0:P], zero_sb[:],
                            start=True, stop=False, skip_group_check=True,
                        )

                    # A = G @ Wq^T chunkwise; each chunk feeds the
                    # pair-packed score reduction.  Software-pipelined
                    # one chunk ahead so score matmuls never wait on a
                    # fresh eviction.
                    with (
                        tc.tile_pool(name="Lab", bufs=2) as Lab,
                        tc.tile_pool(name="a_ps", bufs=2, space="PSUM") as aps,
                    ):
                        def emit_a(dc):
                            a_ps = aps.tile([P, D], f32, tag="aps")
                            for jc in range(NKC):
                                for h in range(2):
                                    nc.tensor.matmul(
                                        a_ps[:, _ts(h, 512)],
                                        g_tiles[jc][:, _ts(dc, P)],
                                        wq_sb[:, jc, _ts(h, 512)],
                                        start=(jc == 0), stop=(jc == NKC - 1),
                                    )
                            # evict pre-scaled by 1/sqrt(dh): exact
                            # (power of 2), and it drops the bias-scale
                            # hop from the softmax dependency chain
                            a_sb = Lab.tile([P, D], f32r, tag="ab")
                            nc.scalar.mul(a_sb[:, 0:512], a_ps[:, 0:512], 0.125)
                            nc.vector.tensor_scalar_mul(
                                a_sb[:, 512:D], a_ps[:, 512:D], 0.125)
                            return a_sb

                        def emit_scores(dc, a_sb):
                            for pr in range(NPAIR):
                                nc.tensor.matmul(
                                    scores_ps[:, _ts(pr, 256)],
                                    wk_sb[:, dc, _ts(pr, P)],
                                    a_sb[:, _ts(pr // 2, 256)],
                                    start=False, stop=False,
                                    skip_group_check=True,
                                )

                        prev = (0, emit_a(0))
                        for dc in range(1, NKC):
                            a_sb = emit_a(dc)
                            emit_scores(*prev)
                            prev = (dc, a_sb)
                        emit_scores(*prev)

                    # ---- softmax + DS, interleaved by pair parity e.
                    # The 16 valid (64x64) diag blocks sit at cols
                    # 512k+384e+64hf (pr=2k+e).  One strided 3D-AP
                    # reduce per (e, hf) group yields all 4 per-BLOCK
                    # maxes at once (per-block max subtraction is
                    # mandatory: block maxes within a group differ by
                    # >91, past f32 exp underflow -- a shared group max
                    # NaNs the weak block's denominator).  Scores arrive
                    # pre-scaled by 0.125 so the max feeds exp's bias
                    # directly; exp's accum_out emits each block's
                    # denominator for free.  Parity 0's DS matmuls run
                    # on the PE while parity 1's chains are still on
                    # DVE/ACT; the d_ps pool sits on the A-pool PSUM
                    # zone (free at score end), not the scores zone the
                    # exps read.
                    sc3 = scores_ps.rearrange("p (k c) -> p k c", c=512)
                    with tc.tile_pool(name="ds_ps", bufs=2, space="PSUM") as dsps:
                        for e in range(2):
                            for hf in range(2):
                                g = 2 * e + hf
                                rows = slice(64 * hf, 64 * hf + 64)
                                off = 384 * e + 64 * hf
                                nc.vector.reduce_max(
                                    nbm[g][rows, 0:4], sc3[rows, :, off:off + 64],
                                    axis=X, negate=True)
                            for k in range(4):
                                pr = 2 * k + e
                                for hf in range(2):
                                    g = 2 * e + hf
                                    rows = slice(64 * hf, 64 * hf + 64)
                                    off = 512 * k + 384 * e + 64 * hf
                                    nc.scalar.activation(
                                        p_tmp[rows, _ts(pr, 64)],
                                        scores_ps[rows, off:off + 64], EXP,
                                        bias=nbm[g][rows, k:k + 1],
                                        accum_out=den[g][rows, k:k + 1])
                            for k in range(4):
                                pr = 2 * k + e
                                for hf in range(2):
                                    g = 2 * e + hf
                                    rows = slice(64 * hf, 64 * hf + 64)
                                    nc.vector.reciprocal(
                                        rec[g][rows, k:k + 1],
                                        den[g][rows, k:k + 1])
                                    # normalize on DVE only: the ACT
                                    # queue stays clear for the exps
                                    nc.vector.tensor_scalar_mul(
                                        p_tiles[pr][rows, _ts(hf, 64)],
                                        p_tmp[rows, _ts(pr, 64)],
                                        rec[g][rows, k:k + 1])
                                d_ps = dsps.tile([P, D], f32, tag="dsps")
                                for h in range(2):
                                    nc.tensor.matmul(
                                        d_ps[:, _ts(h, 512)],
                                        p_tiles[pr][:],
                                        wv_sb[:, pr, _ts(h, 512)],
                                        start=True, stop=True,
                                    )
                                nc.scalar.copy(
                                    ds_sb[:, pr, 0:512], d_ps[:, 0:512])
                                nc.vector.tensor_copy(
                                    ds_sb[:, pr, 512:D], d_ps[:, 512:D])

                # ---- phase 4b: M = DS^T Wo^T (scores PSUM banks are
                # free now; m_ps takes them) ----
                with tc.tile_pool(name="m_ps", bufs=2, space="PSUM") as mmps:
                    # prefetch the x^T stream for phase 5 (WAR-gated on
                    # the released xs region, so these overlap phase 4)
                    xt_tiles = []
                    for b in range(NSC // 4):
                        t = Lxt.tile([P, NKC, 512], bf16, tag="xt")
                        nc.gpsimd.dma_start(t[:], xTr[:, :, _ts(b, 512)])
                        xt_tiles.append(t)

                    for dj in range(NKC):
                        m_ps = mmps.tile([P, D], f32, tag="mps")
                        for pr in range(NPAIR):
                            for h in range(2):
                                nc.tensor.matmul(
                                    m_ps[:, _ts(h, 512)],
                                    ds_sb[:, pr, _ts(dj, P)],
                                    wo_sb[:, pr, _ts(h, 512)],
                                    start=(pr == 0), stop=(pr == NPAIR - 1),
                                )
                        nc.scalar.copy(m_tiles[dj][:, 0:512], m_ps[:, 0:512])
                        nc.vector.tensor_copy(m_tiles[dj][:, 512:D], m_ps[:, 512:D])

              # ---- phase 5: out = x @ M ----
              with (
                  tc.tile_pool(name="Lob", bufs=2) as Lob,
                  tc.tile_pool(name="o_ps", bufs=1, space="PSUM") as ops,
              ):
                  # a never-written 4-bank placeholder pushes the real
                  # slots past the M pool's banks (whose evictions are
                  # still in flight when the out phase starts) onto the
                  # DS pool's long-freed ones
                  ops.tile([P, D], f32, tag="opspad", name="opspad")
                  for st in range(NSC):
                      xt = xt_tiles[st // 4]
                      so = st % 4
                      o_ps = ops.tile([P, D], f32, tag=f"ops{(st + 1) % 3}",
                                      name=f"opsb{(st + 1) % 3}")
                      for dc in range(NKC):
                          for h in range(2):
                              nc.tensor.matmul(
                                  o_ps[:, _ts(h, 512)],
                                  xt[:, dc, _ts(so, P)],
                                  m_tiles[dc][:, _ts(h, 512)],
                                  start=(dc == 0), stop=(dc == NKC - 1),
                              )
                      o_sb = Lob.tile([P, D], f32, tag="ob")
                      if st < NSC - 1:
                          nc.scalar.copy(o_sb[:, 0:512], o_ps[:, 0:512])
                          nc.vector.tensor_copy(o_sb[:, 512:D], o_ps[:, 512:D])
                          nc.sync.dma_start(out_d.ap()[_ts(st, P), :], o_sb[:])
                      else:
                          # split the last tile so the end-of-program
                          # drain only waits on a 256-col evict + DMA
                          for q in range(4):
                              cs = _ts(q, 256)
                              if q % 2 == 0:
                                  nc.scalar.copy(o_sb[:, cs], o_ps[:, cs])
                              else:
                                  nc.vector.tensor_copy(o_sb[:, cs], o_ps[:, cs])
                              nc.sync.dma_start(
                                  out_d.ap()[_ts(st, P), 256 * q:256 * q + 256],
                                  o_sb[:, cs])

    nc.compile()
    return nc


def _get_program():
    global _PROGRAM
    if _PROGRAM is None:
        _PROGRAM = _build_program()
    return _PROGRAM


def _prep_in_maps(x, Wq, Wk, Wv, Wo):
    import ml_dtypes

    bf = ml_dtypes.bfloat16
    x_np = np.asarray(x, np.float32)
    wqT = np.ascontiguousarray(np.asarray(Wq, np.float32).T)
    wkT = np.ascontiguousarray(np.asarray(Wk, np.float32).T)
    wv = np.ascontiguousarray(np.asarray(Wv, np.float32)).astype(bf)
    woT = np.ascontiguousarray(np.asarray(Wo, np.float32).T).astype(bf)
    eye = np.eye(P, dtype=np.float32)
    in_maps = []
    for b in range(N_CORES):
        xs = np.ascontiguousarray(x_np[b])
        xTb = np.ascontiguousarray(x_np[b].T).astype(bf)
        in_maps.append({"xs": xs, "xTb": xTb, "wqT": wqT, "wkT": wkT,
                        "wv": wv, "woT": woT, "eye": eye})
    return in_maps


def kernel(x, Wq, Wk, Wv, Wo):
    from concourse import bass_utils

    nc = _get_program()
    in_maps = _prep_in_maps(x, Wq, Wk, Wv, Wo)
    res = bass_utils.run_bass_kernel_spmd(nc, in_maps, core_ids=list(range(N_CORES)))
    return np.stack([res.results[b]["out"] for b in range(N_CORES)], axis=0)


# revision 63
# speedup vs baseline: 4.4192x; 1.2239x over previous
"""Trainium2 Bass kernel for nn_Attention_89670327206161.

Dense transformer attention block, B=8 S=4096 D=1024 H=16 (dh=64), fp32.
The reference contracts attention scores over the *sequence* axis:
    scores_h = K_h^T Q_h / sqrt(dh)   -> (dh, dh) per head
    P_h      = softmax(scores_h, axis=-1)
    out_h    = V_h @ P_h              -> (S, dh)
    out      = concat_h(out_h) @ Wo^T

Because P_h is position-independent, the whole pipeline collapses
algebraically (exactly, no approximation):
    G        = x^T x                      (1024x1024 Gram, symmetric)
    scores_h = Wk_h G Wq_h^T              (== K_h^T Q_h)
    M        = Wv^T blockdiag(P_h) Wo^T   (1024x1024)
    out      = x @ M

This does ~10.9e9 MACs/core instead of ~17.8e9 for the direct
projection route (Q/K/V/O GEMMs): G (symmetric-half) + A = G Wq^T +
pair-packed Wk reduction + small M build + one output GEMM.

Sharding: pure data parallelism over batch -- one batch element per
NeuronCore, no collectives.

dtypes: score path (x_seq, G, Wq, Wk, A, softmax) is fp32/f32r --
logits reach |142| so they need ~1e-4 relative accuracy.  The output
path (P, Wv, Wo^T, DS, M, x^T) is bf16: its ~0.3% relative error is
40x under the 2e-2 gate and halves DMA+SBUF there.

Phases (per core):
  1. G = x^T x: stream 32 seq-chunks of 128 in 4 superchunks; PSUM
     accumulates upper-triangular row-block strips (pass B cols
     512:1024 for rc 0..7, pass A cols 0:512 for rc 0..3), DVE adds
     into SBUF G; 22 lower blocks mirrored via PE transpose.
  2. A = G @ Wq^T chunkwise (PSUM->SBUF), each chunk immediately
     reduced into persistent pair-packed score PSUM via Wk^T.
  3. Per-head softmax (max-subtracted exp, row-normalized) -> block-
     diagonal P pairs (bf16).
  4. DS = blockdiag(P)^T-applied Wv rows; M = DS^T-reduce with Wo^T,
     cast bf16.
  5. out = x @ M: stream x^T bf16 in 8 seq-blocks, 32 output tiles,
     DMA to HBM.
"""

import numpy as np

HEADS = 16
B, S, D = 8, 4096, 1024
P = 128                  # partitions
NKC = D // P             # 8 feature chunks of 128
NSC = S // P             # 32 seq chunks of 128
SUPER = 8                # seq chunks per superchunk
NSUP = NSC // SUPER      # 4
NPAIR = HEADS // 2       # 8 head pairs -> 128-wide blocks
N_CORES = 8

# G row-block strips: (rc, c0, c1).  Pass B covers cols 512:1024,
# pass A cols 0:512.  rc3/rc7 take full 512-wide strips (same PE cost
# as the 128-wide remnant at the <256 fp32r penalty) so their lower
# blocks come out directly and need no mirror.
G_PASS_B = [(0, 512, 1024), (1, 512, 1024), (2, 512, 1024), (3, 512, 1024),
            (4, 512, 1024), (5, 640, 1024), (6, 768, 1024), (7, 512, 1024)]
G_PASS_A = [(0, 0, 512), (1, 128, 512), (2, 256, 512), (3, 0, 512)]
# lower-triangle blocks (r, c) still needing a transpose-mirror.
# Blocks whose source strip comes from pass B (r >= 4, source strip c)
# go first in c-major order: strip c's super-3 evictions land in rc
# order, so the PE's transposes start with minimal waiting; the three
# pass-A-sourced mirrors trail.
G_MIRRORS = ([(r, c) for c in range(7) for r in range(max(4, c + 1), NKC)
              if not (r == 7 and 4 <= c < 7)]
             + [(1, 0), (2, 0), (2, 1)])

_PROGRAM = None


def _ts(i, n):
    return slice(i * n, (i + 1) * n)


def _build_program(repeat=1):
    # repeat>1 unrolls the whole computation R times in one program --
    # only used by measurement scripts to amplify device time above the
    # axon RPC dispatch noise.  kernel() always uses repeat=1.
    import concourse.bacc as bacc
    import concourse.mybir as mybir
    import concourse.tile as tile

    f32 = mybir.dt.float32
    f32r = mybir.dt.float32r
    bf16 = mybir.dt.bfloat16
    EXP = mybir.ActivationFunctionType.Exp
    X = mybir.AxisListType.X

    nc = bacc.Bacc(trn_type="TRN2", debug=False, num_devices=N_CORES)

    xs_d = nc.dram_tensor("xs", [S, D], f32r, kind="ExternalInput")
    xT_d = nc.dram_tensor("xTb", [D, S], bf16, kind="ExternalInput")
    wqT_d = nc.dram_tensor("wqT", [D, D], f32r, kind="ExternalInput")
    wkT_d = nc.dram_tensor("wkT", [D, D], f32r, kind="ExternalInput")
    wv_d = nc.dram_tensor("wv", [D, D], bf16, kind="ExternalInput")
    woT_d = nc.dram_tensor("woT", [D, D], bf16, kind="ExternalInput")
    eye_d = nc.dram_tensor("eye", [P, P], f32r, kind="ExternalInput")
    out_d = nc.dram_tensor("out", [S, D], f32, kind="ExternalOutput")

    xs_ap = xs_d.ap()                                        # (4096, 1024)
    xTr = xT_d.ap().rearrange("(c p) s -> p c s", p=P)       # (128, 8, 4096)
    wqTr = wqT_d.ap().rearrange("(c p) m -> p c m", p=P)
    wkTr = wkT_d.ap().rearrange("(c p) m -> p c m", p=P)
    wvr = wv_d.ap().rearrange("(c p) d -> p c d", p=P)
    woTr = woT_d.ap().rearrange("(c p) j -> p c j", p=P)

    with tile.TileContext(nc) as tc:
     for _rep in range(repeat):
      with tc.tile_pool(name="L0", bufs=1) as L0:
        zero_sb = L0.tile([P, 512], f32r, tag="zero")
        eye_sb = L0.tile([P, P], f32r, tag="eye")
        m_tiles = [L0.tile([P, D], bf16, tag=f"m{dj}", name=f"m{dj}")
                   for dj in range(NKC)]
        nc.vector.memset(zero_sb[:].bitcast(f32), 0.0)
        # prewarm the ACT function table during the initial DMA wait --
        # the first activation otherwise pays a ~1.3us LoadActFuncSet
        # right where the mirror copies start
        actw = L0.tile([P, 1], f32, tag="actw")
        nc.scalar.activation(actw[:], zero_sb[:, 0:1], EXP)

        with tc.tile_pool(name="Lg", bufs=1) as Lg:
          # one tile per (row-block, column-half): dependency tracking
          # is tile-granular, so pass-B consumers (all 19 lower-mirror
          # sources) never wait on the final pass-A evictions
          g_lo = [Lg.tile([P, 512], f32r, tag=f"glo{rc}", name=f"glo{rc}")
                  for rc in range(NKC)]
          g_hi = [Lg.tile([P, 512], f32r, tag=f"ghi{rc}", name=f"ghi{rc}")
                  for rc in range(NKC)]

          def g_block(jc, dc):
              # (128 x 128) AP of G[jc*128:(jc+1)*128, dc*128:(dc+1)*128]
              if dc < 4:
                  return g_lo[jc][:, _ts(dc, P)]
              return g_hi[jc][:, _ts(dc - 4, P)]
          with tc.tile_pool(name="Lwq", bufs=1) as Lwq:
            wq_sb = Lwq.tile([P, NKC, D], f32r, tag="wq")

            # HAM warm-up: spin the PE on zero matmuls (gated only on
            # the memset) while the first x chunks are still in flight.
            with tc.tile_pool(name="scr_ps", bufs=1, space="PSUM") as scr:
                w_ps = scr.tile([P, 512], f32, tag="w")
                for _ in range(8):
                    nc.tensor.matmul(
                        w_ps[:], zero_sb[:, 0:P], zero_sb[:],
                        start=True, stop=False, skip_group_check=True,
                    )

            # ---- phase 1: G = x^T x (upper triangle) ----
            # DMA issue order matters: the x stream is the critical
            # path at startup, so three superchunks queue ahead of any
            # weight transfer; wq is issued after super 3's fetch.
            with (
                tc.tile_pool(name="Lxs", bufs=3 * SUPER) as Lxs,
                tc.tile_pool(name="g_ps", bufs=1, space="PSUM") as gps,
            ):
                def fetch_super(sp):
                    ts = []
                    for i in range(SUPER):
                        t = Lxs.tile([P, D], f32r, tag="xs")
                        nc.sync.dma_start(
                            t[:], xs_ap[_ts(sp * SUPER + i, P), :])
                        ts.append(t)
                    return ts

                # Bank (tag) plan: pass B tags = (rc+4)%8, pass A tags
                # = rc.  Within each pass, matmuls are emitted for the
                # longest-freed banks first and evictions run in the
                # order the *next* pass needs its banks back, so the PE
                # never waits more than ~0.3us on a DVE eviction.
                supers = [fetch_super(0)]
                # eye is not needed until the mirrors (~78us): queue it
                # behind the first superchunk so chunk 0 lands sooner
                nc.sync.dma_start(eye_sb[:], eye_d.ap())
                supers += [fetch_super(1), fetch_super(2)]
                for sp in range(NSUP):
                    xs_tiles = supers[sp]
                    if sp + 3 < NSUP:
                        supers.append(fetch_super(sp + 3))
                    if sp == 0:
                        # on the sync queue AFTER super 3's chunks: the
                        # SWDGE queue would fire it immediately and the
                        # (serialized) DMA engines would stall the
                        # critical x stream behind a 4MB transfer
                        nc.sync.dma_start(wq_sb[:], wqTr)
                    for strips, tag_of, emit_order, evict_order in (
                        (G_PASS_B, lambda rc: (rc + 4) % 8,
                         (0, 1, 2, 3, 4, 5, 6, 7), (4, 5, 6, 7, 0, 1, 2, 3)),
                        (G_PASS_A, lambda rc: rc,
                         (0, 1, 2, 3), (0, 1, 2, 3)),
                    ):
                        by_rc = {rc: (c0, c1) for rc, c0, c1 in strips}
                        ps = {rc: gps.tile([P, 512], f32, tag=f"g{tag_of(rc)}",
                                           name=f"gps{tag_of(rc)}")
                              for rc in emit_order}
                        for i, xt in enumerate(xs_tiles):
                            for rc in emit_order:
                                c0, c1 = by_rc[rc]
                                nc.tensor.matmul(
                                    ps[rc][:, 0:c1 - c0],
                                    xt[:, _ts(rc, P)], xt[:, c0:c1],
                                    start=(i == 0), stop=(i == SUPER - 1),
                                )
                        for rc in evict_order:
                            c0, c1 = by_rc[rc]
                            if c0 >= 512:
                                dst = g_hi[rc][:, c0 - 512:c1 - 512]
                            else:
                                dst = g_lo[rc][:, c0:c1]
                            if sp == 0:
                                nc.vector.tensor_copy(dst, ps[rc][:, 0:c1 - c0])
                            else:
                                nc.vector.tensor_add(
                                    dst, dst, ps[rc][:, 0:c1 - c0])

                # mirror the remaining lower-triangle blocks: all of a
                # target (row, half)'s transposes land side-by-side in
                # one wide PSUM tile, evicted by a single wide copy
                # (split across ACT and DVE) -- 22 narrow copies would
                # make the DVE the bottleneck of this whole phase.  The
                # tiles come from the G PSUM pool itself (a separate
                # pool could not open until every G eviction retired);
                # tags g4-g7 hold pass B's rc0-3, whose super-3 reads
                # finish during the final pass-A matmuls, so the first
                # transposes start immediately.  All sources of the 19
                # early mirrors live in g_hi (pass B evictions).
                groups = {}
                for r, c in G_MIRRORS:
                    groups.setdefault((r, c // 4), []).append(c)
                mir_tag = {(4, 0): 4, (5, 0): 5, (6, 0): 6, (7, 0): 7,
                           (5, 1): 0, (6, 1): 1, (1, 0): 2, (2, 0): 3}
                mwide = {key: gps.tile([P, 512], f32, tag=f"g{t}",
                                       name=f"mir{t}")
                         for key, t in mir_tag.items()}
                for r, c in G_MIRRORS:
                    key = (r, c // 4)
                    i = groups[key].index(c)
                    nc.tensor.matmul(
                        mwide[key][:, _ts(i, P)].bitcast(f32r),
                        g_block(c, r), eye_sb[:],
                        is_transpose=True,
                        start=True, stop=True, skip_group_check=True)
                for j, key in enumerate([(4, 0), (5, 0), (6, 0), (7, 0),
                                         (5, 1), (6, 1), (1, 0), (2, 0)]):
                    r, half = key
                    cs = groups[key]
                    c0 = min(cs) - 4 * half
                    gt = g_lo[r] if half == 0 else g_hi[r]
                    dst = gt[:, c0 * P:(c0 + len(cs)) * P]
                    src = mwide[key][:, 0:len(cs) * P]
                    if j % 2 == 0:
                        nc.scalar.copy(dst, src)
                    else:
                        nc.vector.tensor_copy(dst, src)

            # ---- phases 2-4 ----
            with tc.tile_pool(name="Lxt", bufs=3) as Lxt:
              with tc.tile_pool(name="L3", bufs=1) as L3:
                # allocation order fixes the SBUF address each tile
                # reuses from the released xs zone, and with it the
                # WAR release time of its DMA: wk (needed first, at the
                # score reduction) goes LAST so it sits over the xs
                # slots whose readers finish two passes early.
                wv_sb = L3.tile([P, NKC, D], bf16, tag="wv")
                wo_sb = L3.tile([P, NKC, D], bf16, tag="wo")
                # per-pair blockdiag P tiles so DS matmuls unblock as
                # soon as their own pair's softmax lands
                p_tiles = []
                for pr in range(NPAIR):
                    pt = L3.tile([P, P], bf16, tag=f"p{pr}", name=f"p{pr}")
                    nc.vector.memset(pt[:], 0.0)
                    p_tiles.append(pt)
                # softmax scratch (lives outside the scores-PSUM scope:
                # normalize/divide run after scores_ps is released).
                # One tile per (e, hf) group -- dependency tracking is
                # tile-granular, so shared tiles would serialize every
                # group's chain behind the last writer.
                p_tmp = L3.tile([P, 512], f32, tag="ptmp")
                nbm = [L3.tile([P, 4], f32, tag=f"nbm{g}", name=f"nbm{g}")
                       for g in range(4)]
                den = [L3.tile([P, 4], f32, tag=f"den{g}", name=f"den{g}")
                       for g in range(4)]
                rec = [L3.tile([P, 4], f32, tag=f"rec{g}", name=f"rec{g}")
                       for g in range(4)]
                ds_tiles = [L3.tile([P, D], bf16, tag=f"ds{pr}",
                                    name=f"dsb{pr}") for pr in range(NPAIR)]
                wk_sb = L3.tile([P, NKC, D], f32r, tag="wk")
                nc.gpsimd.dma_start(wk_sb[:], wkTr)
                nc.gpsimd.dma_start(wv_sb[:], wvr)
                nc.gpsimd.dma_start(wo_sb[:], woTr)

                with tc.tile_pool(name="sc_ps", bufs=1, space="PSUM") as scps:
                  if True:
                    scores_ps = scps.tile([P, NPAIR * 256], f32, tag="sc")
                    for i in range(4):
                        nc.tensor.matmul(
                            scores_ps[:, _ts(i, 512)],
                            zero_sb[:, 0:P], zero_sb[:],
                            start=True, stop=False, skip_group_check=True,
                        )

                    # A = G @ Wq^T chunkwise; each chunk feeds the
                    # pair-packed score reduction.  Software-pipelined
                    # one chunk ahead so score matmuls never wait on a
                    # fresh eviction.
                    with (
                        tc.tile_pool(name="Lab", bufs=2) as Lab,
                        tc.tile_pool(name="a_ps", bufs=2, space="PSUM") as aps,
                    ):
                        def emit_a(dc):
                            a_ps = [aps.tile([P, 512], f32, tag=f"aps{h}",
                                             name=f"aps{h}")
                                    for h in range(2)]
                            for jc in range(NKC):
                                for h in range(2):
                                    nc.tensor.matmul(
                                        a_ps[h][:],
                                        g_block(jc, dc),
                                        wq_sb[:, jc, _ts(h, 512)],
                                        start=(jc == 0), stop=(jc == NKC - 1),
                                    )
                            # evict pre-scaled by 1/sqrt(dh): exact
                            # (power of 2), and it drops the bias-scale
                            # hop from the softmax dependency chain
                            a_sb = Lab.tile([P, D], f32r, tag="ab")
                            nc.scalar.mul(a_sb[:, 0:512], a_ps[0][:], 0.125)
                            nc.vector.tensor_scalar_mul(
                                a_sb[:, 512:D], a_ps[1][:], 0.125)
                            return a_sb

                        def emit_scores(dc, a_sb):
                            for pr in range(NPAIR):
                                nc.tensor.matmul(
                                    scores_ps[:, _ts(pr, 256)],
                                    wk_sb[:, dc, _ts(pr, P)],
                                    a_sb[:, _ts(pr // 2, 256)],
                                    start=False, stop=False,
                                    skip_group_check=True,
                                )

                        prev = (0, emit_a(0))
                        for dc in range(1, NKC):
                            a_sb = emit_a(dc)
                            emit_scores(*prev)
                            prev = (dc, a_sb)
                        emit_scores(*prev)

                    # ---- softmax head: the 16 valid (64x64) diag
                    # blocks sit at cols 512k+384e+64hf (pr=2k+e).  One
                    # strided 3D-AP reduce per (e, hf) group yields all
                    # 4 per-BLOCK maxes at once (per-block max
                    # subtraction is mandatory: block maxes within a
                    # group differ by >91, past f32 exp underflow -- a
                    # shared group max NaNs the weak block's
                    # denominator).  Scores arrive pre-scaled by 0.125
                    # so the max feeds exp's bias directly; exp's
                    # accum_out emits each block's denominator free.
                    sc3 = scores_ps.rearrange("p (k c) -> p k c", c=512)
                    for e in range(2):
                        for hf in range(2):
                            g = 2 * e + hf
                            rows = slice(64 * hf, 64 * hf + 64)
                            off = 384 * e + 64 * hf
                            nc.vector.reduce_max(
                                nbm[g][rows, 0:4], sc3[rows, :, off:off + 64],
                                axis=X, negate=True)
                        for k in range(4):
                            pr = 2 * k + e
                            for hf in range(2):
                                g = 2 * e + hf
                                rows = slice(64 * hf, 64 * hf + 64)
                                off = 512 * k + 384 * e + 64 * hf
                                nc.scalar.activation(
                                    p_tmp[rows, _ts(pr, 64)],
                                    scores_ps[rows, off:off + 64], EXP,
                                    bias=nbm[g][rows, k:k + 1],
                                    accum_out=den[g][rows, k:k + 1])
                  # normalize + DS (PSUM from the freed A-pool zone)
                  with tc.tile_pool(name="ds_ps", bufs=2, space="PSUM") as dsps:
                   for e in range(2):
                    for k in range(4):
                        pr = 2 * k + e
                        for hf in range(2):
                            g = 2 * e + hf
                            rows = slice(64 * hf, 64 * hf + 64)
                            nc.vector.reciprocal(
                                rec[g][rows, k:k + 1],
                                den[g][rows, k:k + 1])
                            # normalize on DVE only: the ACT queue
                            # stays clear for the exps
                            nc.vector.tensor_scalar_mul(
                                p_tiles[pr][rows, _ts(hf, 64)],
                                p_tmp[rows, _ts(pr, 64)],
                                rec[g][rows, k:k + 1])
                        d_ps = [dsps.tile([P, 512], f32, tag=f"ds{h}",
                                          name=f"dsp{h}")
                                for h in range(2)]
                        for h in range(2):
                            nc.tensor.matmul(
                                d_ps[h][:],
                                p_tiles[pr][:],
                                wv_sb[:, pr, _ts(h, 512)],
                                start=True, stop=True,
                            )
                        nc.scalar.copy(
                            ds_tiles[pr][:, 0:512], d_ps[0][:])
                        nc.vector.tensor_copy(
                            ds_tiles[pr][:, 512:D], d_ps[1][:])

                # ---- phase 4b: M = DS^T Wo^T (on the freed scores
                # banks) ----
                with tc.tile_pool(name="m_ps", bufs=2, space="PSUM") as mmps:
                    # prefetch the x^T stream for phase 5 (WAR-gated on
                    # the released xs region, so these overlap phase 4)
                    xt_tiles = []
                    for b in range(NSC // 4):
                        t = Lxt.tile([P, NKC, 512], bf16, tag="xt")
                        nc.gpsimd.dma_start(t[:], xTr[:, :, _ts(b, 512)])
                        xt_tiles.append(t)

                    # pr accumulation order matches DS completion order
                    pr_order = [0, 2, 4, 6, 1, 3, 5, 7]
                    for dj in range(NKC):
                        m_ps = mmps.tile([P, D], f32, tag="mps")
                        for i, pr in enumerate(pr_order):
                            for h in range(2):
                                nc.tensor.matmul(
                                    m_ps[:, _ts(h, 512)],
                                    ds_tiles[pr][:, _ts(dj, P)],
                                    wo_sb[:, pr, _ts(h, 512)],
                                    start=(i == 0), stop=(i == NPAIR - 1),
                                )
                        nc.scalar.copy(m_tiles[dj][:, 0:512], m_ps[:, 0:512])
                        nc.vector.tensor_copy(m_tiles[dj][:, 512:D], m_ps[:, 512:D])

              # ---- phase 5: out = x @ M ----
              with (
                  tc.tile_pool(name="Lob", bufs=2) as Lob,
                  tc.tile_pool(name="o_ps", bufs=1, space="PSUM") as ops,
              ):
                  # a never-written 4-bank placeholder pushes the real
                  # slots past the M pool's banks (whose evictions are
                  # still in flight when the out phase starts) onto the
                  # DS pool's long-freed ones
                  ops.tile([P, D], f32, tag="opspad", name="opspad")
                  for st in range(NSC):
                      xt = xt_tiles[st // 4]
                      so = st % 4
                      o_ps = ops.tile([P, D], f32, tag=f"ops{(st + 1) % 3}",
                                      name=f"opsb{(st + 1) % 3}")
                      for dc in range(NKC):
                          for h in range(2):
                              nc.tensor.matmul(
                                  o_ps[:, _ts(h, 512)],
                                  xt[:, dc, _ts(so, P)],
                                  m_tiles[dc][:, _ts(h, 512)],
                                  start=(dc == 0), stop=(dc == NKC - 1),
                              )
                      o_sb = Lob.tile([P, D], f32, tag="ob")
                      if st < NSC - 1:
                          nc.scalar.copy(o_sb[:, 0:512], o_ps[:, 0:512])
                          nc.vector.tensor_copy(o_sb[:, 512:D], o_ps[:, 512:D])
                          nc.sync.dma_start(out_d.ap()[_ts(st, P), :], o_sb[:])
                      else:
                          # split the last tile so the end-of-program
                          # drain only waits on a 256-col evict + DMA
                          for q in range(4):
                              cs = _ts(q, 256)
                              if q % 2 == 0:
                                  nc.scalar.copy(o_sb[:, cs], o_ps[:, cs])
                              else:
                                  nc.vector.tensor_copy(o_sb[:, cs], o_ps[:, cs])
                              nc.sync.dma_start(
                                  out_d.ap()[_ts(st, P), 256 * q:256 * q + 256],
                                  o_sb[:, cs])

    nc.compile()
    return nc


def _get_program():
    global _PROGRAM
    if _PROGRAM is None:
        _PROGRAM = _build_program()
    return _PROGRAM


def _prep_in_maps(x, Wq, Wk, Wv, Wo):
    import ml_dtypes

    bf = ml_dtypes.bfloat16
    x_np = np.asarray(x, np.float32)
    wqT = np.ascontiguousarray(np.asarray(Wq, np.float32).T)
    wkT = np.ascontiguousarray(np.asarray(Wk, np.float32).T)
    wv = np.ascontiguousarray(np.asarray(Wv, np.float32)).astype(bf)
    woT = np.ascontiguousarray(np.asarray(Wo, np.float32).T).astype(bf)
    eye = np.eye(P, dtype=np.float32)
    in_maps = []
    for b in range(N_CORES):
        xs = np.ascontiguousarray(x_np[b])
        xTb = np.ascontiguousarray(x_np[b].T).astype(bf)
        in_maps.append({"xs": xs, "xTb": xTb, "wqT": wqT, "wkT": wkT,
                        "wv": wv, "woT": woT, "eye": eye})
    return in_maps


def kernel(x, Wq, Wk, Wv, Wo):
    from concourse import bass_utils

    nc = _get_program()
    in_maps = _prep_in_maps(x, Wq, Wk, Wv, Wo)
    res = bass_utils.run_bass_kernel_spmd(nc, in_maps, core_ids=list(range(N_CORES)))
    return np.stack([res.results[b]["out"] for b in range(N_CORES)], axis=0)


# revision 78
# speedup vs baseline: 4.7294x; 1.0702x over previous
"""Trainium2 Bass kernel for nn_Attention_89670327206161.

Dense transformer attention block, B=8 S=4096 D=1024 H=16 (dh=64), fp32.
The reference contracts attention scores over the *sequence* axis:
    scores_h = K_h^T Q_h / sqrt(dh)   -> (dh, dh) per head
    P_h      = softmax(scores_h, axis=-1)
    out_h    = V_h @ P_h              -> (S, dh)
    out      = concat_h(out_h) @ Wo^T

Because P_h is position-independent, the whole pipeline collapses
algebraically (exactly, no approximation):
    G        = x^T x                      (1024x1024 Gram, symmetric)
    scores_h = Wk_h G Wq_h^T              (== K_h^T Q_h)
    M        = Wv^T blockdiag(P_h) Wo^T   (1024x1024)
    out      = x @ M

This does ~10.9e9 MACs/core instead of ~17.8e9 for the direct
projection route (Q/K/V/O GEMMs): G (symmetric-half) + A = G Wq^T +
pair-packed Wk reduction + small M build + one output GEMM.

Sharding: pure data parallelism over batch -- one batch element per
NeuronCore, no collectives.

dtypes: score path (x_seq, G, Wq, Wk, A, softmax) is fp32/f32r --
logits reach |142| so they need ~1e-4 relative accuracy.  The output
path (P, Wv, Wo^T, DS, M, x^T) is bf16: its ~0.3% relative error is
40x under the 2e-2 gate and halves DMA+SBUF there.

Phases (per core):
  1. G = x^T x: stream 32 seq-chunks of 128 in 4 superchunks; PSUM
     accumulates upper-triangular row-block strips (pass B cols
     512:1024 for rc 0..7, pass A cols 0:512 for rc 0..3), DVE adds
     into SBUF G; 22 lower blocks mirrored via PE transpose.
  2. A = G @ Wq^T chunkwise (PSUM->SBUF), each chunk immediately
     reduced into persistent pair-packed score PSUM via Wk^T.
  3. Per-head softmax (max-subtracted exp, row-normalized) -> block-
     diagonal P pairs (bf16).
  4. DS = blockdiag(P)^T-applied Wv rows; M = DS^T-reduce with Wo^T,
     cast bf16.
  5. out = x @ M: stream x^T bf16 in 8 seq-blocks, 32 output tiles,
     DMA to HBM.
"""

import numpy as np

HEADS = 16
B, S, D = 8, 4096, 1024
P = 128                  # partitions
NKC = D // P             # 8 feature chunks of 128
NSC = S // P             # 32 seq chunks of 128
SUPER = 8                # seq chunks per superchunk
NSUP = NSC // SUPER      # 4
NPAIR = HEADS // 2       # 8 head pairs -> 128-wide blocks
N_CORES = 8

# G row-block strips: (rc, c0, c1).  Pass B covers cols 512:1024,
# pass A cols 0:512.  rc3/rc7 take full 512-wide strips (same PE cost
# as the 128-wide remnant at the <256 fp32r penalty) so their lower
# blocks come out directly and need no mirror.
G_PASS_B = [(0, 512, 1024), (1, 512, 1024), (2, 512, 1024), (3, 512, 1024),
            (4, 512, 1024), (5, 640, 1024), (6, 768, 1024), (7, 512, 1024)]
G_PASS_A = [(0, 0, 512), (1, 128, 512), (2, 256, 512), (3, 0, 512)]
# lower-triangle blocks (r, c) still needing a transpose-mirror.
# Blocks whose source strip comes from pass B (r >= 4, source strip c)
# go first in c-major order: strip c's super-3 evictions land in rc
# order, so the PE's transposes start with minimal waiting; the three
# pass-A-sourced mirrors trail.
G_MIRRORS = ([(r, c) for c in range(7) for r in range(max(4, c + 1), NKC)
              if not (r == 7 and 4 <= c < 7)]
             + [(1, 0), (2, 0), (2, 1)])

_PROGRAM = None


def _ts(i, n):
    return slice(i * n, (i + 1) * n)


def _build_program(repeat=1):
    # repeat>1 unrolls the whole computation R times in one program --
    # only used by measurement scripts to amplify device time above the
    # axon RPC dispatch noise.  kernel() always uses repeat=1.
    import concourse.bacc as bacc
    import concourse.mybir as mybir
    import concourse.tile as tile

    f32 = mybir.dt.float32
    f32r = mybir.dt.float32r
    bf16 = mybir.dt.bfloat16
    EXP = mybir.ActivationFunctionType.Exp
    X = mybir.AxisListType.X

    nc = bacc.Bacc(trn_type="TRN2", debug=False, num_devices=N_CORES)

    xs_d = nc.dram_tensor("xs", [S, D], f32r, kind="ExternalInput")
    xT_d = nc.dram_tensor("xTb", [D, S], bf16, kind="ExternalInput")
    wqT_d = nc.dram_tensor("wqT", [D, D], f32r, kind="ExternalInput")
    wkT_d = nc.dram_tensor("wkT", [D, D], f32r, kind="ExternalInput")
    wv_d = nc.dram_tensor("wv", [D, D], bf16, kind="ExternalInput")
    woT_d = nc.dram_tensor("woT", [D, D], bf16, kind="ExternalInput")
    eye_d = nc.dram_tensor("eye", [P, P], f32r, kind="ExternalInput")
    out_d = nc.dram_tensor("out", [S, D], f32, kind="ExternalOutput")

    xs_ap = xs_d.ap()                                        # (4096, 1024)
    xTr = xT_d.ap().rearrange("(c p) s -> p c s", p=P)       # (128, 8, 4096)
    wqTr = wqT_d.ap().rearrange("(c p) m -> p c m", p=P)
    wkTr = wkT_d.ap().rearrange("(c p) m -> p c m", p=P)
    wvr = wv_d.ap().rearrange("(c p) d -> p c d", p=P)
    woTr = woT_d.ap().rearrange("(c p) j -> p c j", p=P)

    with tile.TileContext(nc) as tc:
     for _rep in range(repeat):
      with tc.tile_pool(name="L0", bufs=1) as L0:
        zero_sb = L0.tile([P, 512], f32r, tag="zero")
        eye_sb = L0.tile([P, P], f32r, tag="eye")
        m_tiles = [L0.tile([P, D], bf16, tag=f"m{dj}", name=f"m{dj}")
                   for dj in range(NKC)]
        nc.vector.memset(zero_sb[:].bitcast(f32), 0.0)
        # prewarm the ACT function table during the initial DMA wait --
        # the first activation otherwise pays a ~1.3us LoadActFuncSet
        # right where the mirror copies start
        actw = L0.tile([P, 1], f32, tag="actw")
        nc.scalar.activation(actw[:], zero_sb[:, 0:1], EXP)

        with tc.tile_pool(name="Lg", bufs=1) as Lg:
          # one tile per (row-block, column-half): dependency tracking
          # is tile-granular, so pass-B consumers (all 19 lower-mirror
          # sources) never wait on the final pass-A evictions
          g_lo = [Lg.tile([P, 512], f32r, tag=f"glo{rc}", name=f"glo{rc}")
                  for rc in range(NKC)]
          g_hi = [Lg.tile([P, 512], f32r, tag=f"ghi{rc}", name=f"ghi{rc}")
                  for rc in range(NKC)]

          def g_block(jc, dc):
              # (128 x 128) AP of G[jc*128:(jc+1)*128, dc*128:(dc+1)*128]
              if dc < 4:
                  return g_lo[jc][:, _ts(dc, P)]
              return g_hi[jc][:, _ts(dc - 4, P)]
          with tc.tile_pool(name="Lwq", bufs=1) as Lwq:
            wq_sb = Lwq.tile([P, NKC, D], f32r, tag="wq")

            # HAM warm-up: spin the PE on zero matmuls (gated only on
            # the memset) while the first x chunks are still in flight.
            with tc.tile_pool(name="scr_ps", bufs=1, space="PSUM") as scr:
                w_ps = scr.tile([P, 512], f32, tag="w")
                for _ in range(8):
                    nc.tensor.matmul(
                        w_ps[:], zero_sb[:, 0:P], zero_sb[:],
                        start=True, stop=False, skip_group_check=True,
                    )

            # ---- phase 1: G = x^T x (upper triangle) ----
            # DMA issue order matters: the x stream is the critical
            # path at startup, so three superchunks queue ahead of any
            # weight transfer; wq is issued after super 3's fetch.
            with (
                tc.tile_pool(name="Lxs", bufs=3 * SUPER) as Lxs,
                tc.tile_pool(name="g_ps", bufs=1, space="PSUM") as gps,
            ):
                def fetch_super(sp):
                    ts = []
                    for i in range(SUPER):
                        t = Lxs.tile([P, D], f32r, tag="xs")
                        nc.sync.dma_start(
                            t[:], xs_ap[_ts(sp * SUPER + i, P), :])
                        ts.append(t)
                    return ts

                # Bank (tag) plan: pass B tags = (rc+4)%8, pass A tags
                # = rc.  Within each pass, matmuls are emitted for the
                # longest-freed banks first and evictions run in the
                # order the *next* pass needs its banks back, so the PE
                # never waits more than ~0.3us on a DVE eviction.
                supers = [fetch_super(0)]
                # eye is not needed until the mirrors (~78us): queue it
                # behind the first superchunk so chunk 0 lands sooner
                nc.sync.dma_start(eye_sb[:], eye_d.ap())
                supers += [fetch_super(1), fetch_super(2)]
                for sp in range(NSUP):
                    xs_tiles = supers[sp]
                    if sp + 3 < NSUP:
                        supers.append(fetch_super(sp + 3))
                    if sp == 0:
                        # on the sync queue AFTER super 3's chunks: the
                        # SWDGE queue would fire it immediately and the
                        # (serialized) DMA engines would stall the
                        # critical x stream behind a 4MB transfer
                        nc.sync.dma_start(wq_sb[:], wqTr)
                    for strips, tag_of, emit_order, evict_order in (
                        (G_PASS_B, lambda rc: (rc + 4) % 8,
                         (0, 1, 2, 3, 4, 5, 6, 7), (4, 5, 6, 7, 0, 1, 2, 3)),
                        (G_PASS_A, lambda rc: rc,
                         (0, 1, 2, 3), (0, 1, 2, 3)),
                    ):
                        by_rc = {rc: (c0, c1) for rc, c0, c1 in strips}
                        ps = {rc: gps.tile([P, 512], f32, tag=f"g{tag_of(rc)}",
                                           name=f"gps{tag_of(rc)}")
                              for rc in emit_order}
                        for i, xt in enumerate(xs_tiles):
                            for rc in emit_order:
                                c0, c1 = by_rc[rc]
                                nc.tensor.matmul(
                                    ps[rc][:, 0:c1 - c0],
                                    xt[:, _ts(rc, P)], xt[:, c0:c1],
                                    start=(i == 0), stop=(i == SUPER - 1),
                                )
                        for rc in evict_order:
                            c0, c1 = by_rc[rc]
                            if c0 >= 512:
                                dst = g_hi[rc][:, c0 - 512:c1 - 512]
                            else:
                                dst = g_lo[rc][:, c0:c1]
                            if sp == 0:
                                nc.vector.tensor_copy(dst, ps[rc][:, 0:c1 - c0])
                            else:
                                nc.vector.tensor_add(
                                    dst, dst, ps[rc][:, 0:c1 - c0])

                # mirror the remaining lower-triangle blocks: all of a
                # target (row, half)'s transposes land side-by-side in
                # one wide PSUM tile, evicted by a single wide copy
                # (split across ACT and DVE) -- 22 narrow copies would
                # make the DVE the bottleneck of this whole phase.  The
                # tiles come from the G PSUM pool itself (a separate
                # pool could not open until every G eviction retired);
                # tags g4-g7 hold pass B's rc0-3, whose super-3 reads
                # finish during the final pass-A matmuls, so the first
                # transposes start immediately.  All sources of the 19
                # early mirrors live in g_hi (pass B evictions).
                groups = {}
                for r, c in G_MIRRORS:
                    groups.setdefault((r, c // 4), []).append(c)
                mir_tag = {(4, 0): 4, (5, 0): 5, (6, 0): 6, (7, 0): 7,
                           (5, 1): 0, (6, 1): 1, (1, 0): 2, (2, 0): 3}
                mwide = {key: gps.tile([P, 512], f32, tag=f"g{t}",
                                       name=f"mir{t}")
                         for key, t in mir_tag.items()}
                for r, c in G_MIRRORS:
                    key = (r, c // 4)
                    i = groups[key].index(c)
                    nc.tensor.matmul(
                        mwide[key][:, _ts(i, P)].bitcast(f32r),
                        g_block(c, r), eye_sb[:],
                        is_transpose=True,
                        start=True, stop=True, skip_group_check=True)
                for j, key in enumerate([(4, 0), (5, 0), (6, 0), (7, 0),
                                         (5, 1), (6, 1), (1, 0), (2, 0)]):
                    r, half = key
                    cs = groups[key]
                    c0 = min(cs) - 4 * half
                    gt = g_lo[r] if half == 0 else g_hi[r]
                    dst = gt[:, c0 * P:(c0 + len(cs)) * P]
                    src = mwide[key][:, 0:len(cs) * P]
                    if j % 2 == 0:
                        nc.scalar.copy(dst, src)
                    else:
                        nc.vector.tensor_copy(dst, src)

            # ---- phases 2-4 ----
            with tc.tile_pool(name="Lxt", bufs=3) as Lxt:
              with tc.tile_pool(name="L3", bufs=1) as L3:
                # allocation order fixes the SBUF address each tile
                # reuses from the released xs zone, and with it the
                # WAR release time of its DMA: wk (needed first, at the
                # score reduction) goes LAST so it sits over the xs
                # slots whose readers finish two passes early.
                wv_sb = L3.tile([P, NKC, D], bf16, tag="wv")
                wo_sb = L3.tile([P, NKC, D], bf16, tag="wo")
                # per-pair blockdiag P tiles so DS matmuls unblock as
                # soon as their own pair's softmax lands
                p_tiles = []
                for pr in range(NPAIR):
                    pt = L3.tile([P, P], bf16, tag=f"p{pr}", name=f"p{pr}")
                    nc.vector.memset(pt[:], 0.0)
                    p_tiles.append(pt)
                # softmax scratch (lives outside the scores-PSUM scope:
                # normalize/divide run after scores_ps is released).
                # One tile per (e, hf) group -- dependency tracking is
                # tile-granular, so shared tiles would serialize every
                # group's chain behind the last writer.
                p_tmp = L3.tile([P, 512], f32, tag="ptmp")
                nbm = [L3.tile([P, 4], f32, tag=f"nbm{g}", name=f"nbm{g}")
                       for g in range(4)]
                den = [L3.tile([P, 4], f32, tag=f"den{g}", name=f"den{g}")
                       for g in range(4)]
                rec = [L3.tile([P, 4], f32, tag=f"rec{g}", name=f"rec{g}")
                       for g in range(4)]
                ds_tiles = [L3.tile([P, D], bf16, tag=f"ds{pr}",
                                    name=f"dsb{pr}") for pr in range(NPAIR)]
                wk_sb = L3.tile([P, NKC, D], f32r, tag="wk")
                nc.gpsimd.dma_start(wk_sb[:], wkTr)
                nc.gpsimd.dma_start(wv_sb[:], wvr)
                nc.gpsimd.dma_start(wo_sb[:], woTr)

                with tc.tile_pool(name="sc_ps", bufs=1, space="PSUM") as scps:
                  if True:
                    # scores split by pair parity: even pairs in one
                    # tile, odd in the other -- parity 0's softmax head
                    # stops waiting on the final odd-pair matmuls, and
                    # the valid diag blocks land on a regular stride.
                    sc_par = [scps.tile([P, 1024], f32, tag=f"sc{e}",
                                        name=f"sc{e}") for e in range(2)]
                    for e in range(2):
                        for i in range(2):
                            nc.tensor.matmul(
                                sc_par[e][:, _ts(i, 512)],
                                zero_sb[:, 0:P], zero_sb[:],
                                start=True, stop=False, skip_group_check=True,
                            )

                    # A = G @ Wq^T chunkwise; each chunk feeds the
                    # pair-packed score reduction.  Software-pipelined
                    # one chunk ahead so score matmuls never wait on a
                    # fresh eviction.
                    with (
                        tc.tile_pool(name="Lab", bufs=2) as Lab,
                        tc.tile_pool(name="a_ps", bufs=2, space="PSUM") as aps,
                    ):
                        def emit_a(dc):
                            a_ps = [aps.tile([P, 512], f32, tag=f"aps{h}",
                                             name=f"aps{h}")
                                    for h in range(2)]
                            for jc in range(NKC):
                                for h in range(2):
                                    nc.tensor.matmul(
                                        a_ps[h][:],
                                        g_block(jc, dc),
                                        wq_sb[:, jc, _ts(h, 512)],
                                        start=(jc == 0), stop=(jc == NKC - 1),
                                    )
                            # evict pre-scaled by 1/sqrt(dh): exact
                            # (power of 2), and it drops the bias-scale
                            # hop from the softmax dependency chain
                            a_sb = Lab.tile([P, D], f32r, tag="ab")
                            nc.scalar.mul(a_sb[:, 0:512], a_ps[0][:], 0.125)
                            nc.vector.tensor_scalar_mul(
                                a_sb[:, 512:D], a_ps[1][:], 0.125)
                            return a_sb

                        def emit_scores(dc, a_sb):
                            for pr in (0, 2, 4, 6, 1, 3, 5, 7):
                                nc.tensor.matmul(
                                    sc_par[pr % 2][:, _ts(pr // 2, 256)],
                                    wk_sb[:, dc, _ts(pr, P)],
                                    a_sb[:, _ts(pr // 2, 256)],
                                    start=False, stop=False,
                                    skip_group_check=True,
                                )

                        prev = (0, emit_a(0))
                        for dc in range(1, NKC):
                            a_sb = emit_a(dc)
                            emit_scores(*prev)
                            prev = (dc, a_sb)
                        emit_scores(*prev)

                    # ---- softmax head: the 16 valid (64x64) diag
                    # blocks sit at cols 512k+384e+64hf (pr=2k+e).  One
                    # strided 3D-AP reduce per (e, hf) group yields all
                    # 4 per-BLOCK maxes at once (per-block max
                    # subtraction is mandatory: block maxes within a
                    # group differ by >91, past f32 exp underflow -- a
                    # shared group max NaNs the weak block's
                    # denominator).  Scores arrive pre-scaled by 0.125
                    # so the max feeds exp's bias directly; exp's
                    # accum_out emits each block's denominator free.
                    for e in range(2):
                        sc3 = sc_par[e].rearrange("p (k c) -> p k c", c=256)
                        for hf in range(2):
                            g = 2 * e + hf
                            rows = slice(64 * hf, 64 * hf + 64)
                            off = 128 * e + 64 * hf
                            nc.vector.reduce_max(
                                nbm[g][rows, 0:4], sc3[rows, :, off:off + 64],
                                axis=X, negate=True)
                        for k in range(4):
                            pr = 2 * k + e
                            for hf in range(2):
                                g = 2 * e + hf
                                rows = slice(64 * hf, 64 * hf + 64)
                                off = 256 * k + 128 * e + 64 * hf
                                nc.scalar.activation(
                                    p_tmp[rows, _ts(pr, 64)],
                                    sc_par[e][rows, off:off + 64], EXP,
                                    bias=nbm[g][rows, k:k + 1],
                                    accum_out=den[g][rows, k:k + 1])
                  # normalize + DS (PSUM from the freed A-pool zone)
                  with tc.tile_pool(name="ds_ps", bufs=2, space="PSUM") as dsps:
                   for e in range(2):
                    for k in range(4):
                        pr = 2 * k + e
                        for hf in range(2):
                            g = 2 * e + hf
                            rows = slice(64 * hf, 64 * hf + 64)
                            nc.vector.reciprocal(
                                rec[g][rows, k:k + 1],
                                den[g][rows, k:k + 1])
                            # normalize on DVE only: the ACT queue
                            # stays clear for the exps (gpsimd and ACT
                            # variants both measured slower)
                            nc.vector.tensor_scalar_mul(
                                p_tiles[pr][rows, _ts(hf, 64)],
                                p_tmp[rows, _ts(pr, 64)],
                                rec[g][rows, k:k + 1])
                        d_ps = [dsps.tile([P, 512], f32, tag=f"ds{h}",
                                          name=f"dsp{h}")
                                for h in range(2)]
                        for h in range(2):
                            nc.tensor.matmul(
                                d_ps[h][:],
                                p_tiles[pr][:],
                                wv_sb[:, pr, _ts(h, 512)],
                                start=True, stop=True,
                            )
                        nc.scalar.copy(
                            ds_tiles[pr][:, 0:512], d_ps[0][:])
                        nc.vector.tensor_copy(
                            ds_tiles[pr][:, 512:D], d_ps[1][:])

                # ---- phase 4b: M = DS^T Wo^T (on the freed scores
                # banks) ----
                with tc.tile_pool(name="m_ps", bufs=2, space="PSUM") as mmps:
                    # prefetch the x^T stream for phase 5 (WAR-gated on
                    # the released xs region, so these overlap phase 4)
                    xt_tiles = []
                    for b in range(NSC // 4):
                        t = Lxt.tile([P, NKC, 512], bf16, tag="xt")
                        nc.gpsimd.dma_start(t[:], xTr[:, :, _ts(b, 512)])
                        xt_tiles.append(t)

                    # pr accumulation order matches DS completion order
                    pr_order = [0, 2, 4, 6, 1, 3, 5, 7]
                    for dj in range(NKC):
                        m_ps = mmps.tile([P, D], f32, tag="mps")
                        for i, pr in enumerate(pr_order):
                            for h in range(2):
                                nc.tensor.matmul(
                                    m_ps[:, _ts(h, 512)],
                                    ds_tiles[pr][:, _ts(dj, P)],
                                    wo_sb[:, pr, _ts(h, 512)],
                                    start=(i == 0), stop=(i == NPAIR - 1),
                                )
                        nc.scalar.copy(m_tiles[dj][:, 0:512], m_ps[:, 0:512])
                        nc.vector.tensor_copy(m_tiles[dj][:, 512:D], m_ps[:, 512:D])

                    # ---- phase 5: out = x @ M, sharing the M pool's
                    # PSUM slots: a separate pool could not open until
                    # every M eviction retired (pool-boundary barrier),
                    # stalling the first out tile ~1.7us.  Slot reuse
                    # WARs only against the dj6 eviction, which retires
                    # during dj7's matmuls.  ds0/ds1's freed banks add a
                    # third slot for deeper pipelining.
                    with tc.tile_pool(name="Lob", bufs=2) as Lob:
                      for st in range(NSC):
                        xt = xt_tiles[st // 4]
                        so = st % 4
                        o_ps = mmps.tile([P, D], f32, tag="ops", name="opsm")
                        for dc in range(NKC):
                            for h in range(2):
                                nc.tensor.matmul(
                                    o_ps[:, _ts(h, 512)],
                                    xt[:, dc, _ts(so, P)],
                                    m_tiles[dc][:, _ts(h, 512)],
                                    start=(dc == 0), stop=(dc == NKC - 1),
                                )
                        o_sb = Lob.tile([P, D], f32, tag="ob")
                        if st < NSC - 1:
                            nc.scalar.copy(o_sb[:, 0:512], o_ps[:, 0:512])
                            nc.vector.tensor_copy(o_sb[:, 512:D], o_ps[:, 512:D])
                            nc.sync.dma_start(out_d.ap()[_ts(st, P), :], o_sb[:])
                        else:
                            # split the last tile so the end-of-program
                            # drain only waits on a 256-col evict + DMA
                            for q in range(4):
                                cs = _ts(q, 256)
                                if q % 2 == 0:
                                    nc.scalar.copy(o_sb[:, cs], o_ps[:, cs])
                                else:
                                    nc.vector.tensor_copy(o_sb[:, cs], o_ps[:, cs])
                                nc.sync.dma_start(
                                    out_d.ap()[_ts(st, P), 256 * q:256 * q + 256],
                                    o_sb[:, cs])

    nc.compile()
    return nc


def _get_program():
    global _PROGRAM
    if _PROGRAM is None:
        _PROGRAM = _build_program()
    return _PROGRAM


def _prep_in_maps(x, Wq, Wk, Wv, Wo):
    import ml_dtypes

    bf = ml_dtypes.bfloat16
    x_np = np.asarray(x, np.float32)
    wqT = np.ascontiguousarray(np.asarray(Wq, np.float32).T)
    wkT = np.ascontiguousarray(np.asarray(Wk, np.float32).T)
    wv = np.ascontiguousarray(np.asarray(Wv, np.float32)).astype(bf)
    woT = np.ascontiguousarray(np.asarray(Wo, np.float32).T).astype(bf)
    eye = np.eye(P, dtype=np.float32)
    in_maps = []
    for b in range(N_CORES):
        xs = np.ascontiguousarray(x_np[b])
        xTb = np.ascontiguousarray(x_np[b].T).astype(bf)
        in_maps.append({"xs": xs, "xTb": xTb, "wqT": wqT, "wkT": wkT,
                        "wv": wv, "woT": woT, "eye": eye})
    return in_maps


def kernel(x, Wq, Wk, Wv, Wo):
    from concourse import bass_utils

    nc = _get_program()
    in_maps = _prep_in_maps(x, Wq, Wk, Wv, Wo)
    res = bass_utils.run_bass_kernel_spmd(nc, in_maps, core_ids=list(range(N_CORES)))
    return np.stack([res.results[b]["out"] for b in range(N_CORES)], axis=0)


# revision 86
# speedup vs baseline: 5.0116x; 1.0597x over previous
"""Trainium2 Bass kernel for nn_Attention_89670327206161.

Dense transformer attention block, B=8 S=4096 D=1024 H=16 (dh=64), fp32.
The reference contracts attention scores over the *sequence* axis:
    scores_h = K_h^T Q_h / sqrt(dh)   -> (dh, dh) per head
    P_h      = softmax(scores_h, axis=-1)
    out_h    = V_h @ P_h              -> (S, dh)
    out      = concat_h(out_h) @ Wo^T

Because P_h is position-independent, the whole pipeline collapses
algebraically (exactly, no approximation):
    G        = x^T x                      (1024x1024 Gram, symmetric)
    scores_h = Wk_h G Wq_h^T              (== K_h^T Q_h)
    M        = Wv^T blockdiag(P_h) Wo^T   (1024x1024)
    out      = x @ M

This does ~10.9e9 MACs/core instead of ~17.8e9 for the direct
projection route (Q/K/V/O GEMMs): G (symmetric-half) + A = G Wq^T +
pair-packed Wk reduction + small M build + one output GEMM.

Sharding: pure data parallelism over batch -- one batch element per
NeuronCore, no collectives.

dtypes: score path (x_seq, G, Wq, Wk, A, softmax) is fp32/f32r --
logits reach |142| so they need ~1e-4 relative accuracy.  The output
path (P, Wv, Wo^T, DS, M, x^T) is bf16: its ~0.3% relative error is
40x under the 2e-2 gate and halves DMA+SBUF there.

Phases (per core):
  1. G = x^T x: stream 32 seq-chunks of 128 in 4 superchunks; PSUM
     accumulates upper-triangular row-block strips (pass B cols
     512:1024 for rc 0..7, pass A cols 0:512 for rc 0..3), DVE adds
     into SBUF G; 22 lower blocks mirrored via PE transpose.
  2. A = G @ Wq^T chunkwise (PSUM->SBUF), each chunk immediately
     reduced into persistent pair-packed score PSUM via Wk^T.
  3. Per-head softmax (max-subtracted exp, row-normalized) -> block-
     diagonal P pairs (bf16).
  4. DS = blockdiag(P)^T-applied Wv rows; M = DS^T-reduce with Wo^T,
     cast bf16.
  5. out = x @ M: stream x^T bf16 in 8 seq-blocks, 32 output tiles,
     DMA to HBM.
"""

import numpy as np

HEADS = 16
B, S, D = 8, 4096, 1024
P = 128                  # partitions
NKC = D // P             # 8 feature chunks of 128
NSC = S // P             # 32 seq chunks of 128
SUPER = 8                # seq chunks per superchunk
NSUP = NSC // SUPER      # 4
NPAIR = HEADS // 2       # 8 head pairs -> 128-wide blocks
N_CORES = 8

# G row-block strips: (rc, c0, c1).  Pass B covers cols 512:1024,
# pass A cols 0:512.  rc3/rc7 take full 512-wide strips (same PE cost
# as the 128-wide remnant at the <256 fp32r penalty) so their lower
# blocks come out directly and need no mirror.
G_PASS_B = [(0, 512, 1024), (1, 512, 1024), (2, 512, 1024), (3, 512, 1024),
            (4, 512, 1024), (5, 640, 1024), (6, 768, 1024), (7, 512, 1024)]
G_PASS_A = [(0, 0, 512), (1, 128, 512), (2, 256, 512), (3, 0, 512)]
# lower-triangle blocks (r, c) still needing a transpose-mirror.
# Blocks whose source strip comes from pass B (r >= 4, source strip c)
# go first in c-major order: strip c's super-3 evictions land in rc
# order, so the PE's transposes start with minimal waiting; the three
# pass-A-sourced mirrors trail.
G_MIRRORS = ([(r, c) for c in range(7) for r in range(max(4, c + 1), NKC)
              if not (r == 7 and 4 <= c < 7)]
             + [(1, 0), (2, 0), (2, 1)])

_PROGRAM = None


def _ts(i, n):
    return slice(i * n, (i + 1) * n)


def _build_program(repeat=1):
    # repeat>1 unrolls the whole computation R times in one program --
    # only used by measurement scripts to amplify device time above the
    # axon RPC dispatch noise.  kernel() always uses repeat=1.
    import concourse.bacc as bacc
    import concourse.mybir as mybir
    import concourse.tile as tile

    f32 = mybir.dt.float32
    f32r = mybir.dt.float32r
    bf16 = mybir.dt.bfloat16
    EXP = mybir.ActivationFunctionType.Exp
    X = mybir.AxisListType.X

    nc = bacc.Bacc(trn_type="TRN2", debug=False, num_devices=N_CORES)

    xs_d = nc.dram_tensor("xs", [S, D], f32r, kind="ExternalInput")
    xT_d = nc.dram_tensor("xTb", [D, S], bf16, kind="ExternalInput")
    wqT_d = nc.dram_tensor("wqT", [D, D], f32r, kind="ExternalInput")
    wkT_d = nc.dram_tensor("wkT", [D, D], f32r, kind="ExternalInput")
    wv_d = nc.dram_tensor("wv", [D, D], bf16, kind="ExternalInput")
    woT_d = nc.dram_tensor("woT", [D, D], bf16, kind="ExternalInput")
    eye_d = nc.dram_tensor("eye", [P, P], f32r, kind="ExternalInput")
    out_d = nc.dram_tensor("out", [S, D], f32, kind="ExternalOutput")

    xs_ap = xs_d.ap()                                        # (4096, 1024)
    xTr = xT_d.ap().rearrange("(c p) s -> p c s", p=P)       # (128, 8, 4096)
    wqTr = wqT_d.ap().rearrange("(c p) m -> p c m", p=P)
    wkTr = wkT_d.ap().rearrange("(c p) m -> p c m", p=P)
    wvr = wv_d.ap().rearrange("(c p) d -> p c d", p=P)
    woTr = woT_d.ap().rearrange("(c p) j -> p c j", p=P)

    with tile.TileContext(nc) as tc:
     for _rep in range(repeat):
      with tc.tile_pool(name="L0", bufs=1) as L0:
        zero_sb = L0.tile([P, 512], f32r, tag="zero")
        eye_sb = L0.tile([P, P], f32r, tag="eye")
        m_tiles = [L0.tile([P, D], bf16, tag=f"m{dj}", name=f"m{dj}")
                   for dj in range(NKC)]
        nc.vector.memset(zero_sb[:].bitcast(f32), 0.0)
        # prewarm the ACT function table during the initial DMA wait --
        # the first activation otherwise pays a ~1.3us LoadActFuncSet
        # right where the mirror copies start
        actw = L0.tile([P, 1], f32, tag="actw")
        nc.scalar.activation(actw[:], zero_sb[:, 0:1], EXP)

        with tc.tile_pool(name="Lg", bufs=1) as Lg:
          # one tile per (row-block, column-half): dependency tracking
          # is tile-granular, so pass-B consumers (all 19 lower-mirror
          # sources) never wait on the final pass-A evictions
          g_lo = [Lg.tile([P, 512], f32r, tag=f"glo{rc}", name=f"glo{rc}")
                  for rc in range(NKC)]
          g_hi = [Lg.tile([P, 512], f32r, tag=f"ghi{rc}", name=f"ghi{rc}")
                  for rc in range(NKC)]

          def g_block(jc, dc):
              # (128 x 128) AP of G[jc*128:(jc+1)*128, dc*128:(dc+1)*128]
              if dc < 4:
                  return g_lo[jc][:, _ts(dc, P)]
              return g_hi[jc][:, _ts(dc - 4, P)]
          with tc.tile_pool(name="Lwq", bufs=1) as Lwq:
            wq_sb = Lwq.tile([P, NKC, D], f32r, tag="wq")

            # HAM warm-up: spin the PE on zero matmuls (gated only on
            # the memset) while the first x chunks are still in flight.
            with tc.tile_pool(name="scr_ps", bufs=1, space="PSUM") as scr:
                w_ps = scr.tile([P, 512], f32, tag="w")
                for _ in range(8):
                    nc.tensor.matmul(
                        w_ps[:], zero_sb[:, 0:P], zero_sb[:],
                        start=True, stop=False, skip_group_check=True,
                    )

            # ---- phase 1: G = x^T x (upper triangle) ----
            # DMA issue order matters: the x stream is the critical
            # path at startup, so three superchunks queue ahead of any
            # weight transfer; wq is issued after super 3's fetch.
            with (
                tc.tile_pool(name="Lxs", bufs=3 * SUPER) as Lxs,
                tc.tile_pool(name="g_ps", bufs=1, space="PSUM") as gps,
            ):
                def fetch_super(sp):
                    ts = []
                    for i in range(SUPER):
                        t = Lxs.tile([P, D], f32r, tag="xs")
                        nc.sync.dma_start(
                            t[:], xs_ap[_ts(sp * SUPER + i, P), :])
                        ts.append(t)
                    return ts

                # Bank (tag) plan: pass B tags = (rc+4)%8, pass A tags
                # = rc.  Within each pass, matmuls are emitted for the
                # longest-freed banks first and evictions run in the
                # order the *next* pass needs its banks back, so the PE
                # never waits more than ~0.3us on a DVE eviction.
                supers = [fetch_super(0)]
                # eye is not needed until the mirrors (~78us): queue it
                # behind the first superchunk so chunk 0 lands sooner
                nc.sync.dma_start(eye_sb[:], eye_d.ap())
                supers += [fetch_super(1), fetch_super(2)]
                for sp in range(NSUP):
                    xs_tiles = supers[sp]
                    if sp + 3 < NSUP:
                        supers.append(fetch_super(sp + 3))
                    if sp == 0:
                        # on the sync queue AFTER super 3's chunks: the
                        # SWDGE queue would fire it immediately and the
                        # (serialized) DMA engines would stall the
                        # critical x stream behind a 4MB transfer
                        nc.sync.dma_start(wq_sb[:], wqTr)
                    for strips, tag_of, emit_order, evict_order in (
                        (G_PASS_B, lambda rc: (rc + 4) % 8,
                         (0, 1, 2, 3, 4, 5, 6, 7), (4, 5, 6, 7, 0, 1, 2, 3)),
                        (G_PASS_A, lambda rc: rc,
                         (0, 1, 2, 3), (0, 1, 2, 3)),
                    ):
                        by_rc = {rc: (c0, c1) for rc, c0, c1 in strips}
                        ps = {rc: gps.tile([P, 512], f32, tag=f"g{tag_of(rc)}",
                                           name=f"gps{tag_of(rc)}")
                              for rc in emit_order}
                        for i, xt in enumerate(xs_tiles):
                            for rc in emit_order:
                                c0, c1 = by_rc[rc]
                                nc.tensor.matmul(
                                    ps[rc][:, 0:c1 - c0],
                                    xt[:, _ts(rc, P)], xt[:, c0:c1],
                                    start=(i == 0), stop=(i == SUPER - 1),
                                )
                        for rc in evict_order:
                            c0, c1 = by_rc[rc]
                            if c0 >= 512:
                                dst = g_hi[rc][:, c0 - 512:c1 - 512]
                            else:
                                dst = g_lo[rc][:, c0:c1]
                            if sp == 0:
                                nc.vector.tensor_copy(dst, ps[rc][:, 0:c1 - c0])
                            else:
                                nc.vector.tensor_add(
                                    dst, dst, ps[rc][:, 0:c1 - c0])

                # mirror the remaining lower-triangle blocks: all of a
                # target (row, half)'s transposes land side-by-side in
                # one wide PSUM tile, evicted by a single wide copy
                # (split across ACT and DVE) -- 22 narrow copies would
                # make the DVE the bottleneck of this whole phase.  The
                # tiles come from the G PSUM pool itself (a separate
                # pool could not open until every G eviction retired);
                # tags g4-g7 hold pass B's rc0-3, whose super-3 reads
                # finish during the final pass-A matmuls, so the first
                # transposes start immediately.  All sources of the 19
                # early mirrors live in g_hi (pass B evictions).
                groups = {}
                for r, c in G_MIRRORS:
                    groups.setdefault((r, c // 4), []).append(c)
                mir_tag = {(4, 0): 4, (5, 0): 5, (6, 0): 6, (7, 0): 7,
                           (5, 1): 0, (6, 1): 1, (1, 0): 2, (2, 0): 3}
                mwide = {key: gps.tile([P, 512], f32, tag=f"g{t}",
                                       name=f"mir{t}")
                         for key, t in mir_tag.items()}
                for r, c in G_MIRRORS:
                    key = (r, c // 4)
                    i = groups[key].index(c)
                    nc.tensor.matmul(
                        mwide[key][:, _ts(i, P)].bitcast(f32r),
                        g_block(c, r), eye_sb[:],
                        is_transpose=True,
                        start=True, stop=True, skip_group_check=True)
                for j, key in enumerate([(4, 0), (5, 0), (6, 0), (7, 0),
                                         (5, 1), (6, 1), (1, 0), (2, 0)]):
                    r, half = key
                    cs = groups[key]
                    c0 = min(cs) - 4 * half
                    gt = g_lo[r] if half == 0 else g_hi[r]
                    dst = gt[:, c0 * P:(c0 + len(cs)) * P]
                    src = mwide[key][:, 0:len(cs) * P]
                    if j % 2 == 0:
                        nc.scalar.copy(dst, src)
                    else:
                        nc.vector.tensor_copy(dst, src)

            # ---- phases 2-4 ----
            with tc.tile_pool(name="Lxt", bufs=3) as Lxt:
              with tc.tile_pool(name="L3", bufs=1) as L3:
                # allocation order fixes the SBUF address each tile
                # reuses from the released xs zone, and with it the
                # WAR release time of its DMA: wk (needed first, at the
                # score reduction) goes LAST so it sits over the xs
                # slots whose readers finish two passes early.
                wv_sb = L3.tile([P, NKC, D], bf16, tag="wv")
                wo_sb = L3.tile([P, NKC, D], bf16, tag="wo")
                # per-pair blockdiag P tiles so DS matmuls unblock as
                # soon as their own pair's softmax lands
                p_tiles = []
                for pr in range(NPAIR):
                    pt = L3.tile([P, P], bf16, tag=f"p{pr}", name=f"p{pr}")
                    nc.vector.memset(pt[:], 0.0)
                    p_tiles.append(pt)
                # softmax scratch (lives outside the scores-PSUM scope:
                # normalize/divide run after scores_ps is released).
                # One tile per (e, hf) group -- dependency tracking is
                # tile-granular, so shared tiles would serialize every
                # group's chain behind the last writer.
                p_tmp = L3.tile([P, 512], f32, tag="ptmp")
                nbm = [L3.tile([P, 4], f32, tag=f"nbm{g}", name=f"nbm{g}")
                       for g in range(4)]
                den = [L3.tile([P, 4], f32, tag=f"den{g}", name=f"den{g}")
                       for g in range(4)]
                rec = [L3.tile([P, 4], f32, tag=f"rec{g}", name=f"rec{g}")
                       for g in range(4)]
                ds_tiles = [L3.tile([P, D], bf16, tag=f"ds{pr}",
                                    name=f"dsb{pr}") for pr in range(NPAIR)]
                wk_sb = L3.tile([P, NKC, D], f32r, tag="wk")
                nc.gpsimd.dma_start(wk_sb[:], wkTr)
                nc.gpsimd.dma_start(wv_sb[:], wvr)
                nc.gpsimd.dma_start(wo_sb[:], woTr)

                with tc.tile_pool(name="sc_ps", bufs=1, space="PSUM") as scps:
                  if True:
                    # scores split by pair parity: even pairs in one
                    # tile, odd in the other -- parity 0's softmax head
                    # stops waiting on the final odd-pair matmuls, and
                    # the valid diag blocks land on a regular stride.
                    sc_par = [scps.tile([P, 1024], f32, tag=f"sc{e}",
                                        name=f"sc{e}") for e in range(2)]
                    for e in range(2):
                        for i in range(2):
                            nc.tensor.matmul(
                                sc_par[e][:, _ts(i, 512)],
                                zero_sb[:, 0:P], zero_sb[:],
                                start=True, stop=False, skip_group_check=True,
                            )

                    # A = G @ Wq^T chunkwise; each chunk feeds the
                    # pair-packed score reduction.  Software-pipelined
                    # one chunk ahead so score matmuls never wait on a
                    # fresh eviction.
                    with (
                        tc.tile_pool(name="Lab", bufs=2) as Lab,
                        tc.tile_pool(name="a_ps", bufs=2, space="PSUM") as aps,
                    ):
                        def emit_a(dc):
                            a_ps = [aps.tile([P, 512], f32, tag=f"aps{h}",
                                             name=f"aps{h}")
                                    for h in range(2)]
                            for jc in range(NKC):
                                for h in range(2):
                                    nc.tensor.matmul(
                                        a_ps[h][:],
                                        g_block(jc, dc),
                                        wq_sb[:, jc, _ts(h, 512)],
                                        start=(jc == 0), stop=(jc == NKC - 1),
                                    )
                            # evict pre-scaled by 1/sqrt(dh): exact
                            # (power of 2), and it drops the bias-scale
                            # hop from the softmax dependency chain
                            a_sb = Lab.tile([P, D], f32r, tag="ab")
                            nc.scalar.mul(a_sb[:, 0:512], a_ps[0][:], 0.125)
                            nc.vector.tensor_scalar_mul(
                                a_sb[:, 512:D], a_ps[1][:], 0.125)
                            return a_sb

                        def emit_scores(dc, a_sb):
                            for pr in (0, 2, 4, 6, 1, 3, 5, 7):
                                nc.tensor.matmul(
                                    sc_par[pr % 2][:, _ts(pr // 2, 256)],
                                    wk_sb[:, dc, _ts(pr, P)],
                                    a_sb[:, _ts(pr // 2, 256)],
                                    start=False, stop=False,
                                    skip_group_check=True,
                                )

                        prev = (0, emit_a(0))
                        for dc in range(1, NKC):
                            a_sb = emit_a(dc)
                            emit_scores(*prev)
                            prev = (dc, a_sb)
                        emit_scores(*prev)

                    # ---- softmax head: the 16 valid (64x64) diag
                    # blocks sit at cols 512k+384e+64hf (pr=2k+e).  One
                    # strided 3D-AP reduce per (e, hf) group yields all
                    # 4 per-BLOCK maxes at once (per-block max
                    # subtraction is mandatory: block maxes within a
                    # group differ by >91, past f32 exp underflow -- a
                    # shared group max NaNs the weak block's
                    # denominator).  Scores arrive pre-scaled by 0.125
                    # so the max feeds exp's bias directly; exp's
                    # accum_out emits each block's denominator free.
                    for e in range(2):
                        sc3 = sc_par[e].rearrange("p (k c) -> p k c", c=256)
                        for hf in range(2):
                            g = 2 * e + hf
                            rows = slice(64 * hf, 64 * hf + 64)
                            off = 128 * e + 64 * hf
                            nc.vector.reduce_max(
                                nbm[g][rows, 0:4], sc3[rows, :, off:off + 64],
                                axis=X, negate=True)
                        for k in range(4):
                            pr = 2 * k + e
                            for hf in range(2):
                                g = 2 * e + hf
                                rows = slice(64 * hf, 64 * hf + 64)
                                off = 256 * k + 128 * e + 64 * hf
                                nc.scalar.activation(
                                    p_tmp[rows, _ts(pr, 64)],
                                    sc_par[e][rows, off:off + 64], EXP,
                                    bias=nbm[g][rows, k:k + 1],
                                    accum_out=den[g][rows, k:k + 1])
                  # normalize + DS (PSUM from the freed A-pool zone)
                  with tc.tile_pool(name="ds_ps", bufs=2, space="PSUM") as dsps:
                   for e in range(2):
                    for k in range(4):
                        pr = 2 * k + e
                        for hf in range(2):
                            g = 2 * e + hf
                            rows = slice(64 * hf, 64 * hf + 64)
                            nc.vector.reciprocal(
                                rec[g][rows, k:k + 1],
                                den[g][rows, k:k + 1])
                            # normalize on DVE only: the ACT queue
                            # stays clear for the exps (gpsimd and ACT
                            # variants both measured slower)
                            nc.vector.tensor_scalar_mul(
                                p_tiles[pr][rows, _ts(hf, 64)],
                                p_tmp[rows, _ts(pr, 64)],
                                rec[g][rows, k:k + 1])
                        d_ps = [dsps.tile([P, 512], f32, tag=f"ds{h}",
                                          name=f"dsp{h}")
                                for h in range(2)]
                        for h in range(2):
                            nc.tensor.matmul(
                                d_ps[h][:],
                                p_tiles[pr][:],
                                wv_sb[:, pr, _ts(h, 512)],
                                start=True, stop=True,
                            )
                        nc.scalar.copy(
                            ds_tiles[pr][:, 0:512], d_ps[0][:])
                        nc.vector.tensor_copy(
                            ds_tiles[pr][:, 512:D], d_ps[1][:])

                # ---- phase 4b: M = DS^T Wo^T (on the freed scores
                # banks) ----
                with tc.tile_pool(name="m_ps", bufs=2, space="PSUM") as mmps:
                    # prefetch the x^T stream for phase 5 (WAR-gated on
                    # the released xs region, so these overlap phase 4)
                    xt_tiles = []
                    for b in range(NSC // 4):
                        t = Lxt.tile([P, NKC, 512], bf16, tag="xt")
                        nc.gpsimd.dma_start(t[:], xTr[:, :, _ts(b, 512)])
                        xt_tiles.append(t)

                    # pr accumulation order matches DS completion order
                    pr_order = [0, 2, 4, 6, 1, 3, 5, 7]
                    for dj in range(NKC):
                        m_ps = mmps.tile([P, D], f32, tag="mps")
                        for i, pr in enumerate(pr_order):
                            for h in range(2):
                                nc.tensor.matmul(
                                    m_ps[:, _ts(h, 512)],
                                    ds_tiles[pr][:, _ts(dj, P)],
                                    wo_sb[:, pr, _ts(h, 512)],
                                    start=(i == 0), stop=(i == NPAIR - 1),
                                )
                        nc.scalar.copy(m_tiles[dj][:, 0:512], m_ps[:, 0:512])
                        nc.vector.tensor_copy(m_tiles[dj][:, 512:D], m_ps[:, 512:D])

                    # ---- phase 5: out = x @ M, sharing the M pool's
                    # PSUM slots: a separate pool could not open until
                    # every M eviction retired (pool-boundary barrier),
                    # stalling the first out tile ~1.7us.  Slot reuse
                    # WARs only against the dj6 eviction, which retires
                    # during dj7's matmuls.  ds0/ds1's freed banks add a
                    # third slot for deeper pipelining.
                    with tc.tile_pool(name="Lob", bufs=2) as Lob:
                      for st in range(NSC):
                        xt = xt_tiles[st // 4]
                        so = st % 4
                        o_ps = mmps.tile([P, D], f32, tag="ops", name="opsm")
                        for dc in range(NKC):
                            for h in range(2):
                                nc.tensor.matmul(
                                    o_ps[:, _ts(h, 512)],
                                    xt[:, dc, _ts(so, P)],
                                    m_tiles[dc][:, _ts(h, 512)],
                                    start=(dc == 0), stop=(dc == NKC - 1),
                                )
                        o_sb = Lob.tile([P, D], f32, tag="ob")
                        if st < NSC - 1:
                            nc.scalar.copy(o_sb[:, 0:512], o_ps[:, 0:512])
                            nc.vector.tensor_copy(o_sb[:, 512:D], o_ps[:, 512:D])
                            nc.sync.dma_start(out_d.ap()[_ts(st, P), :], o_sb[:])
                        else:
                            # split the last tile so the end-of-program
                            # drain only waits on a 256-col evict + DMA
                            for q in range(4):
                                cs = _ts(q, 256)
                                if q % 2 == 0:
                                    nc.scalar.copy(o_sb[:, cs], o_ps[:, cs])
                                else:
                                    nc.vector.tensor_copy(o_sb[:, cs], o_ps[:, cs])
                                nc.sync.dma_start(
                                    out_d.ap()[_ts(st, P), 256 * q:256 * q + 256],
                                    o_sb[:, cs])

    nc.compile()
    return nc


def _get_program():
    global _PROGRAM
    if _PROGRAM is None:
        _PROGRAM = _build_program()
    return _PROGRAM


def _prep_in_maps(x, Wq, Wk, Wv, Wo):
    import ml_dtypes

    bf = ml_dtypes.bfloat16
    x_np = np.asarray(x, np.float32)
    wqT = np.ascontiguousarray(np.asarray(Wq, np.float32).T)
    wkT = np.ascontiguousarray(np.asarray(Wk, np.float32).T)
    wv = np.ascontiguousarray(np.asarray(Wv, np.float32)).astype(bf)
    woT = np.ascontiguousarray(np.asarray(Wo, np.float32).T).astype(bf)
    eye = np.eye(P, dtype=np.float32)
    in_maps = []
    for b in range(N_CORES):
        xs = np.ascontiguousarray(x_np[b])
        xTb = np.ascontiguousarray(x_np[b].T).astype(bf)
        in_maps.append({"xs": xs, "xTb": xTb, "wqT": wqT, "wkT": wkT,
                        "wv": wv, "woT": woT, "eye": eye})
    return in_maps


def kernel(x, Wq, Wk, Wv, Wo):
    from concourse import bass_utils

    nc = _get_program()
    in_maps = _prep_in_maps(x, Wq, Wk, Wv, Wo)
    res = bass_utils.run_bass_kernel_spmd(nc, in_maps, core_ids=list(range(N_CORES)))
    return np.stack([res.results[b]["out"] for b in range(N_CORES)], axis=0)
